# revision 32
# baseline (speedup 1.0000x reference)
"""Trainium2 Bass kernel for a Conformer block (B=8, S=1024, D=512).

Sharding: data-parallel over batch — 1 batch element per NeuronCore, 8 cores,
no collectives.

Per-core layout strategy: the residual stream lives in SBUF feature-major
([D, S]) in bf16; every linear layer is then a natural PE matmul with the
stored [in, out] weight as lhsT, all in bf16 (f32 PSUM accumulation) — bf16
keeps the PE out of the fp32-HIGH power-throttle regime. LayerNorm
gains/biases, the attention scale, and BatchNorm are folded into the weights
on the host; x arrives host-transposed ([D, S]) and pre-cast to bf16.
LayerNorm mean/var come from ones-vector matmuls on the PE; mean and rstd
are applied to the matmul input via two K=1 broadcast matmuls plus two
elementwise ops (no augmented-row matmuls); rstd/softmax reciprocals use the
Act-engine Rsqrt/Reciprocal tables (tolerance is loose). Softmax
denominators come for free from a ones column appended to V. The depthwise
conv splits its 31 taps three ways: a middle band as diag-matmul PSUM
accumulation on the otherwise-idle PE, a GPSIMD tail fed by DVE products,
and the rest as DVE scalar_tensor_tensor chains; the conv module is phased
(all GLU sigmoids -> taps -> all SiLUs) to avoid Act table thrash.
"""

import os
import numpy as np

# ---------------- problem constants (hardcoded) ----------------
B, S, D = 8, 1024, 512
H, DH = 8, 64
FFI, CI, KCONV = 1024, 1024, 31
EPS = 1e-5
NCORES = 8
PAD = (KCONV - 1) // 2  # 15
NDC = D // 128    # 4  d-chunks
NTC = S // 128    # 8  t-chunks
HALF = S // 2     # 512

MM_MODE = os.environ.get("CONF_MM_MODE", "bf16")  # bf16 | f32r | f32
SCOPES = os.environ.get("CONF_SCOPES", "1") == "1"
GPS_TAPS = int(os.environ.get("CONF_GPS_TAPS", "4"))  # taps on gpsimd (0:
# the Pool engine rejects AP-scalar ops on this target, so direct GPS taps
# cannot run; the hook remains for a product-fed variant)
PE_TAPS = int(os.environ.get("CONF_PE_TAPS", "16"))  # taps as PE diag matmuls
DBG = os.environ.get("CONF_DEBUG_STAGES", "0") == "1"


# ---------------- tile-framework workaround ----------------
def _patch_tile_drain():
    """This walrus build rejects >1 sync-wait on TPB_CTRL (Drain/NOP)
    instructions; spread the TileContext tail-drain waits across
    single-wait NOPs."""
    import concourse.tile as tile
    from concourse.vector_clock import ScopedClock
    from concourse import mybir

    if getattr(tile.TileContext, "_drain_patched", False):
        return

    def _drain_and_barrier(self, tick_clock, wait_clock):
        nc = self.nc
        carrier = nc.sync.nop(nofuse=True, hint="tail_wait_carrier")
        wait_clock.add_sem_waits(
            carrier.ins, ScopedClock({None: tick_clock.global_clock})
        )
        waits = list(carrier.ins.sync_info.on_wait)
        if len(waits) > 1:
            carrier.ins.sync_info.on_wait = waits[:1]
            for w in waits[1:]:
                nxt = nc.sync.nop(nofuse=True, hint="tail_wait_carrier")
                nxt.ins.sync_info = mybir.SyncInfo(on_wait=[w], on_update=[])
        nc.sync.drain()
        nc.all_engine_barrier()
        assert self.sems is not None
        popped = nc._tile_sem_poison_stack.pop()
        assert popped is self._sem_poison
        nc.clear_and_free_semaphores(list(self.sems.allocated().values()))
        nc.all_engine_barrier()

    tile.TileContext._drain_and_barrier = _drain_and_barrier
    tile.TileContext._drain_patched = True


def _np_mm_dtype():
    import ml_dtypes
    return ml_dtypes.bfloat16 if MM_MODE == "bf16" else np.float32


# ---------------- host-side weight preparation ----------------
def _blob_lhsT(w, nk, nm):
    """[K, M] -> [Mc, 128(p), nk, 128(m)] contiguous (lhsT tile layout)."""
    K, M = w.shape
    assert K == nk * 128 and M == nm * 128
    return np.ascontiguousarray(
        w.reshape(nk, 128, nm, 128).transpose(2, 1, 0, 3)
    ).astype(_np_mm_dtype())


def _blob_bias(b, nm):
    """[M] -> [128, Mc] (per-partition bias columns)."""
    return np.ascontiguousarray(b.reshape(nm, 128).T).astype(np.float32)


def prep_inputs(inp):
    """Fold LN gains/biases, attention scale, BatchNorm, and FF 0.5 scales
    into weights. Returns dict of DRAM arrays shared by all cores."""
    f64 = lambda a: np.asarray(a, np.float64)
    mdt = _np_mm_dtype()
    out = {}

    def ln_matmul_group(pfx, g, lb, w, wb, nm, scale=1.0):
        wg = f64(w) * f64(g)[:, None] * scale
        out[f"{pfx}_w"] = _blob_lhsT(wg, NDC, nm)
        bias = (f64(wb) + f64(lb) @ f64(w)) * scale
        out[f"{pfx}_b"] = _blob_bias(bias, nm)

    # FF1
    ln_matmul_group("ff1a", inp["ff1_ln_g"], inp["ff1_ln_b"],
                    inp["ff1_w1"], inp["ff1_b1"], FFI // 128)
    out["ff1b_w"] = _blob_lhsT(f64(inp["ff1_w2"]) * 0.5, FFI // 128, NDC)
    out["ff1b_b"] = _blob_bias(f64(inp["ff1_b2"]) * 0.5, NDC)

    # attention
    ln_matmul_group("wq", inp["attn_ln_g"], inp["attn_ln_b"],
                    inp["q_w"], inp["q_b"], NDC, scale=DH ** -0.5)
    ln_matmul_group("wk", inp["attn_ln_g"], inp["attn_ln_b"],
                    inp["k_w"], inp["k_b"], NDC)
    # v: rhs layout [kc, p, n]
    wvg = f64(inp["v_w"]) * f64(inp["attn_ln_g"])[:, None]
    out["wv_w"] = np.ascontiguousarray(
        wvg.reshape(NDC, 128, H * DH)
    ).astype(mdt)
    vb = f64(inp["v_b"]) + f64(inp["attn_ln_b"]) @ f64(inp["v_w"])
    out["wv_bias"] = vb.astype(mdt).reshape(1, H * DH)
    out["_has_vb"] = bool(np.abs(vb).max() > 0)
    # o: [Mc, 64(p), H, 128(m)]
    import ml_dtypes
    out["wo_w"] = np.ascontiguousarray(
        f64(inp["o_w"]).reshape(H, DH, NDC, 128).transpose(2, 1, 0, 3)
    ).astype(ml_dtypes.bfloat16)
    out["wo_b"] = _blob_bias(f64(inp["o_b"]), NDC)

    # conv module
    ln_matmul_group("pw1", inp["conv_ln_g"], inp["conv_ln_b"],
                    inp["pw1_w"], inp["pw1_b"], 2 * CI // 128)
    inv = f64(inp["bn_g"]) / np.sqrt(f64(inp["bn_var"]) + EPS)
    dwf = f64(inp["dw_w"])[:, 0, :] * inv[:, None]  # [CI, K]
    out["dw_w"] = np.ascontiguousarray(
        dwf.reshape(CI // 128, 128, KCONV)
    ).astype(np.float32)
    cb = (f64(inp["dw_b"]) - f64(inp["bn_mean"])) * inv + f64(inp["bn_b"])
    out["dw_b"] = _blob_bias(cb, CI // 128)
    out["pw2_w"] = _blob_lhsT(f64(inp["pw2_w"]), CI // 128, NDC)
    out["pw2_b"] = _blob_bias(f64(inp["pw2_b"]), NDC)
    # diag(w[c,j]) lhsT blobs for the PE-matmul taps: [NCC, K, 128, 128]
    n_pe = max(0, min(PE_TAPS, KCONV))
    if n_pe:
        dg = np.zeros((CI // 128, n_pe, 128, 128), np.float64)
        dwr = dwf.reshape(CI // 128, 128, KCONV)
        j0 = (KCONV - n_pe) // 2  # PE takes a middle band of taps
        idx = np.arange(128)
        for pc in range(CI // 128):
            for t in range(n_pe):
                dg[pc, t, idx, idx] = dwr[pc, :, j0 + t]
        out["dw_diag"] = dg.astype(mdt)

    # FF2
    ln_matmul_group("ff2a", inp["ff2_ln_g"], inp["ff2_ln_b"],
                    inp["ff2_w1"], inp["ff2_b1"], FFI // 128)
    out["ff2b_w"] = _blob_lhsT(f64(inp["ff2_w2"]) * 0.5, FFI // 128, NDC)
    out["ff2b_b"] = _blob_bias(f64(inp["ff2_b2"]) * 0.5, NDC)

    # final LN
    out["outln_g"] = _blob_bias(f64(inp["out_ln_g"]), NDC)
    out["outln_b"] = _blob_bias(f64(inp["out_ln_b"]), NDC)
    out["_triv_final"] = bool(
        np.allclose(inp["out_ln_g"], 1.0) and np.allclose(inp["out_ln_b"], 0.0)
    )
    out["ident"] = np.eye(128, dtype=mdt)
    out["ones_c"] = np.ones((128, 128), dtype=mdt)
    return out


# ---------------- kernel builder ----------------
def build_program():
    _patch_tile_drain()
    import concourse.bass as bass
    import concourse.tile as tile
    from concourse import mybir
    from contextlib import ExitStack, nullcontext

    dt = mybir.dt
    AF = mybir.ActivationFunctionType
    OP = mybir.AluOpType
    F32 = dt.float32
    BF16 = dt.bfloat16
    F8 = dt.float8e4
    DR = mybir.MatmulPerfMode.DoubleRow

    MMF = {"bf16": dt.bfloat16, "f32r": dt.float32r,
           "f32": dt.float32}[MM_MODE]

    def mm(ap):
        return ap

    nc = bass.Bass("TRN2", target_bir_lowering=False, debug=False)

    def act_unsafe(out, in_, func, bias=0.0, scale=1.0):
        """Emit an InstActivation bypassing the Reciprocal/Rsqrt accuracy
        guard in bass (tolerance here is 2e-2; the table approximation is
        fine and ~5x faster than the DVE multi-pass reciprocal)."""
        eng = nc.scalar
        inputs = [eng.lower_ap(in_)]
        for arg in (bias, scale, 0.0):
            if isinstance(arg, bass.AP):
                inputs.append(eng.lower_ap(arg))
            else:
                inputs.append(mybir.ImmediateValue(dtype=mybir.dt.float32,
                                                   value=float(arg)))
        return eng.add_instruction(mybir.InstActivation(
            name=nc.get_next_instruction_name(),
            func=func,
            ins=inputs,
            outs=[eng.lower_ap(out)],
        ))

    # ---- DRAM declarations ----
    x_d = nc.dram_tensor("x", [D, S], MMF, kind="ExternalInput")
    y_d = nc.dram_tensor("y", [S, D], F32, kind="ExternalOutput")
    dram = {}

    def din(name, shape, dtp=None):
        dram[name] = nc.dram_tensor(name, list(shape), dtp or F32,
                                    kind="ExternalInput")
        return dram[name]

    def dinm(name, shape):
        return din(name, shape, MMF)

    dinm("ident", [128, 128])
    dinm("ones_c", [128, 128])
    for pfx, nm in [("ff1a", FFI // 128), ("wq", NDC), ("wk", NDC),
                    ("pw1", 2 * CI // 128), ("ff2a", FFI // 128)]:
        dinm(f"{pfx}_w", [nm, 128, NDC, 128])
        din(f"{pfx}_b", [128, nm])
    for pfx, nk, nm in [("ff1b", FFI // 128, NDC), ("pw2", CI // 128, NDC),
                        ("ff2b", FFI // 128, NDC)]:
        dinm(f"{pfx}_w", [nm, 128, nk, 128])
        din(f"{pfx}_b", [128, nm])
    dinm("wv_w", [NDC, 128, H * DH])
    dinm("wv_bias", [1, H * DH])
    N_PE_TAPS = max(0, min(PE_TAPS, KCONV))
    if N_PE_TAPS:
        dinm("dw_diag", [CI // 128, N_PE_TAPS, 128, 128])
    din("wo_w", [NDC, DH, H, 128], BF16)
    din("wo_b", [128, NDC])
    din("dw_w", [CI // 128, 128, KCONV])
    din("dw_b", [128, CI // 128])
    din("outln_g", [128, NDC])
    din("outln_b", [128, NDC])

    dbg_d = {}
    if DBG:
        for s_ in ["ff1", "attn", "conv", "ff2"]:
            dbg_d[s_] = nc.dram_tensor(f"dbg_{s_}", [D, S], MMF,
                                       kind="ExternalOutput")

    HAS_VB = build_program._has_vb
    TRIV_FINAL = build_program._triv_final

    with tile.TileContext(nc) as tc, ExitStack() as top:
        top.enter_context(nc.allow_low_precision(
            reason="bf16 compute is intentional"))
        # ---- global pools ----
        p_x = top.enter_context(tc.tile_pool(name="p_x", bufs=1))
        p_const = top.enter_context(tc.tile_pool(name="p_const", bufs=1))
        p_rows = top.enter_context(tc.tile_pool(name="p_rows", bufs=2))
        p_sq = top.enter_context(tc.tile_pool(name="p_sq", bufs=2))
        p_w = top.enter_context(tc.tile_pool(name="p_w", bufs=3))
        p_wsm = top.enter_context(tc.tile_pool(name="p_wsm", bufs=2))
        p_bias = top.enter_context(tc.tile_pool(name="p_bias", bufs=2))
        p_xs = top.enter_context(tc.tile_pool(name="p_xs", bufs=4))
        p_evt = top.enter_context(tc.tile_pool(name="p_evt", bufs=2))
        # PSUM budget is 8 banks. ps_sc feeds the two MM->ACT ping-pong
        # streams (attention scores->exp, conv tap psums) with enough slots
        # that the producer never waits on the consumer's semaphore
        # round-trip; ps_mm covers everything else; ps_st holds one LN stat.
        ps_mm = top.enter_context(tc.tile_pool(name="ps_mm", bufs=3, space="PSUM"))
        ps_sc = top.enter_context(tc.tile_pool(name="ps_sc", bufs=4, space="PSUM"))
        ps_st = top.enter_context(tc.tile_pool(name="ps_st", bufs=1, space="PSUM"))

        ident = p_const.tile([128, 128], MMF, tag="ident", name="ident")
        nc.sync.dma_start(out=ident, in_=dram["ident"].ap())
        ones = p_const.tile([128, 128], MMF, tag="ones", name="ones")
        nc.sync.dma_start(out=ones, in_=dram["ones_c"].ap())
        epst = p_const.tile([128, 1], F32, tag="epst", name="epst")
        nc.vector.memset(epst, EPS)
        ones8 = p_const.tile([128, 2, 64], F8, tag="ones8", name="ones8")
        nc.vector.memset(ones8, 1.0)

        # ---- load x (already feature-major + cast on host) ----
        x_t = [p_x.tile([128, S], MMF, tag=f"x{i}", name=f"x{i}") for i in range(NDC)]
        xb_t = x_t
        for mc in range(NDC):
            nc.sync.dma_start(out=x_t[mc], in_=x_d[mc * 128:(mc + 1) * 128, :])

        def shadow_cast():
            pass

        # ---- helpers ----
        def ln_stats_rows():
            """LN over feature axis of x_t. Returns xs tiles with
            xs = (x - mean) * rstd (feature-major, centered + scaled)."""
            rstd_b = p_rows.tile([1, S], MMF, tag="rstdb", name="rstdb")
            mrow = p_rows.tile([1, S], MMF, tag="mrow", name="mrow")
            c1 = 1.0 / D
            for th in range(2):
                sl = slice(th * HALF, (th + 1) * HALF)
                s1 = ps_st.tile([1, HALF], F32, tag="st", name="st")
                s2 = ps_mm.tile([1, HALF], F32, tag="ps", name="st2")
                for kc in range(NDC):
                    nc.tensor.matmul(s1, mm(ones[:, 0:1]),
                                     mm(xb_t[kc][:, sl]),
                                     start=(kc == 0), stop=(kc == NDC - 1))
                for kc in range(NDC):
                    sq = p_sq.tile([128, HALF], MMF, tag="sq", name="sq")
                    nc.scalar.square(out=sq, in_=xb_t[kc][:, sl])
                    nc.tensor.matmul(s2, mm(ones[:, 0:1]), mm(sq),
                                     start=(kc == 0), stop=(kc == NDC - 1))
                mean_s = p_rows.tile([1, HALF], F32, tag="tmp",
                                     name="mean_s", bufs=4)
                nc.vector.tensor_scalar_mul(out=mean_s, in0=s1, scalar1=c1)
                msq = p_rows.tile([1, HALF], F32, tag="tmp", name="msq", bufs=4)
                nc.vector.tensor_tensor(out=msq, in0=mean_s, in1=mean_s,
                                        op=OP.mult)
                vpe = p_rows.tile([1, HALF], F32, tag="tmp", name="vpe", bufs=4)
                nc.vector.scalar_tensor_tensor(
                    out=vpe, in0=s2, scalar=c1, in1=msq,
                    op0=OP.mult, op1=OP.subtract)
                # rsqrt via exp(-0.5*ln(v+eps)): ln+exp share one ACT table
                # set (natural_log_exp_and_others), so no table swap against
                # the attention exp / softmax reciprocal path.
                lnv = p_rows.tile([1, HALF], F32, tag="tmp", name="lnv",
                                  bufs=4)
                nc.scalar.activation(out=lnv, in_=vpe, func=AF.Ln,
                                     bias=epst[0:1, :], scale=1.0)
                nc.scalar.activation(out=rstd_b[:, sl], in_=lnv, func=AF.Exp,
                                     bias=0.0, scale=-0.5)
                nc.vector.scalar_tensor_tensor(
                    out=mrow[:, sl], in0=mean_s, scalar=-1.0,
                    in1=rstd_b[:, sl], op0=OP.mult, op1=OP.mult)
            # xs = x * bcast(rstd) + bcast(-mean*rstd); the broadcast tiles
            # are copied to SBUF once (GPSIMD cannot read PSUM) and the
            # per-chunk applies are split DVE/GPSIMD to keep the DVE free.
            xs = [p_xs.tile([128, S], MMF, tag="xs", name="xs") for _ in range(NDC)]
            for th in range(2):
                sl = slice(th * HALF, (th + 1) * HALF)
                pb = ps_mm.tile([128, HALF], F32, tag="ps", name="ps")
                nc.tensor.matmul(pb, mm(ones[0:1, :]), mm(rstd_b[:, sl]),
                                 start=True, stop=True)
                pm = ps_mm.tile([128, HALF], F32, tag="ps", name="ps")
                nc.tensor.matmul(pm, mm(ones[0:1, :]), mm(mrow[:, sl]),
                                 start=True, stop=True)
                pbs = p_evt.tile([128, HALF], BF16, tag="zsg", name="pbs",
                                 bufs=4)
                pms = p_evt.tile([128, HALF], BF16, tag="zsg", name="pms",
                                 bufs=4)
                nc.scalar.copy(out=pbs, in_=pb)
                nc.scalar.copy(out=pms, in_=pm)
                for kc in range(NDC):
                    eng = nc.vector if kc < 2 else nc.gpsimd
                    eng.tensor_tensor(out=xs[kc][:, sl],
                                      in0=x_t[kc][:, sl], in1=pbs,
                                      op=OP.mult)
                    eng.tensor_tensor(out=xs[kc][:, sl],
                                      in0=xs[kc][:, sl], in1=pms,
                                      op=OP.add)
            return xs

        def load_bias(pfx, nm):
            bt = p_bias.tile([128, nm], F32, tag="bias", name="bias")
            nc.sync.dma_start(out=bt, in_=dram[f"{pfx}_b"].ap())
            return bt

        def dense_mm(pfx, nk, nm, rhs_tiles, evict):
            """plain contraction over nk chunks of rhs_tiles."""
            for mc in range(nm):
                wt = p_w.tile([128, nk, 128], MMF, tag="w", name="w")
                nc.sync.dma_start(out=wt, in_=dram[f"{pfx}_w"][mc, :, :, :])
                for th in range(2):
                    sl = slice(th * HALF, (th + 1) * HALF)
                    ps = ps_mm.tile([128, HALF], F32, tag="ps", name="ps")
                    for kc in range(nk):
                        nc.tensor.matmul(ps, mm(wt[:, kc, :]),
                                         mm(rhs_tiles[kc][:, sl]),
                                         start=(kc == 0), stop=(kc == nk - 1))
                    evict(ps, mc, th, sl)

        def dbg_dump(name):
            if DBG:
                for mc in range(NDC):
                    nc.sync.dma_start(
                        out=dbg_d[name][mc * 128:(mc + 1) * 128, :],
                        in_=x_t[mc])

        # ================= FF module =================
        def ff_module(pa, pb):
            xs = ln_stats_rows()
            with tc.tile_pool(name="p_h", bufs=FFI // 128) as p_h:
                bt1 = load_bias(pa, FFI // 128)
                h = [p_h.tile([128, S], MMF, tag="h", name="h") for _ in range(FFI // 128)]

                def ev1(ps, mc, th, sl):
                    nc.scalar.activation(out=h[mc][:, sl], in_=ps, func=AF.Silu,
                                         bias=bt1[:, mc:mc + 1], scale=1.0)

                dense_mm(pa, NDC, FFI // 128, xs, ev1)
                bt2 = load_bias(pb, NDC)

                def ev2(ps, mc, th, sl):
                    nc.vector.scalar_tensor_tensor(
                        out=x_t[mc][:, sl], in0=ps, scalar=bt2[:, mc:mc + 1],
                        in1=x_t[mc][:, sl], op0=OP.add, op1=OP.add)

                dense_mm(pb, FFI // 128, NDC, h, ev2)

        # ================= attention =================
        def attn_module():
            xs = ln_stats_rows()
            with ExitStack() as ph:
                p_qk = ph.enter_context(tc.tile_pool(name="p_qk", bufs=8))
                p_v = ph.enter_context(tc.tile_pool(name="p_v", bufs=NTC))
                p_exp = ph.enter_context(tc.tile_pool(name="p_exp", bufs=24))
                p_ao = ph.enter_context(tc.tile_pool(name="p_ao", bufs=H))
                p_wv = ph.enter_context(tc.tile_pool(name="p_wv", bufs=1))

                q_t = [p_qk.tile([128, S], BF16, tag="qk", name="qk") for _ in range(NDC)]
                k_t = [p_qk.tile([128, S], BF16, tag="qk", name="qk") for _ in range(NDC)]
                btq = load_bias("wq", NDC)
                btk = load_bias("wk", NDC)

                def evq(ps, mc, th, sl):
                    nc.vector.tensor_scalar_add(out=q_t[mc][:, sl], in0=ps,
                                                scalar1=btq[:, mc:mc + 1])

                def evk(ps, mc, th, sl):
                    nc.vector.tensor_scalar_add(out=k_t[mc][:, sl], in0=ps,
                                                scalar1=btk[:, mc:mc + 1])

                dense_mm("wq", NDC, NDC, xs, evq)
                dense_mm("wk", NDC, NDC, xs, evk)

                # v (token-major, with ones column per head)
                wv = p_wv.tile([128, NDC, H * DH], MMF, tag="wv", name="wv")
                nc.sync.dma_start(out=wv, in_=dram["wv_w"].ap().rearrange(
                    "k p n -> p k n"))
                if HAS_VB:
                    wvb = p_wv.tile([1, H * DH], MMF, tag="wvb", name="wvb")
                    nc.sync.dma_start(out=wvb, in_=dram["wv_bias"].ap())
                # v (token-major, with ones column per head)
                v_t = []
                for tck in range(NTC):
                    vt = p_v.tile([128, H, DH + 1], BF16, tag="v", name="v")
                    nc.vector.memset(vt[:, :, DH:DH + 1], 1.0)
                    pv = ps_mm.tile([128, H * DH], F32, tag="ps", name="ps")
                    tsl = slice(tck * 128, (tck + 1) * 128)
                    for kc in range(NDC):
                        nc.tensor.matmul(pv, mm(xs[kc][:, tsl]),
                                         mm(wv[:, kc, :]),
                                         start=(kc == 0),
                                         stop=(kc == NDC - 1 and not HAS_VB))
                    if HAS_VB:
                        nc.tensor.matmul(pv, mm(ones[0:1, :]), mm(wvb),
                                         start=False, stop=True)
                    nc.vector.tensor_copy(
                        out=vt[:, :, 0:DH],
                        in_=pv.rearrange("p (h d) -> p h d", h=H))
                    v_t.append(vt)

                # scores -> exp -> AV -> normalize, software-pipelined with a
                # one-head skew: the PE runs head h+1's score matmuls while
                # the Scalar engine exps head h, so neither engine waits and
                # the PE never idles past the HAM re-throttle window.
                ao_t = [None] * H
                e_heads = [None] * H

                def emit_scores(h_):
                    hp, sub = h_ // 2, h_ % 2
                    base = sub * 64
                    e_t = []
                    for ktc in range(NTC):
                        et = p_exp.tile([128, S], BF16, tag="exp", name="exp")
                        ksl = slice(ktc * 128, (ktc + 1) * 128)
                        for th in range(2):
                            sl = slice(th * HALF, (th + 1) * HALF)
                            pss = ps_sc.tile([128, HALF], F32, tag="sc",
                                             name="sc")
                            nc.tensor.matmul(
                                pss,
                                mm(k_t[hp][base:base + 64, ksl]),
                                mm(q_t[hp][base:base + 64, sl]),
                                start=True, stop=True,
                                tile_position=(base, 0))
                            nc.scalar.activation(out=et[:, sl], in_=pss,
                                                 func=AF.Exp)
                        e_t.append(et)
                    e_heads[h_] = e_t

                pavs_t = [None] * H

                def emit_av_mm(h_):
                    """AV matmuls; pav is copied to SBUF immediately so the
                    psum bank frees within ~0.5us."""
                    e_t = e_heads[h_]
                    pavs = [None, None]
                    for th in range(2):
                        sl = slice(th * HALF, (th + 1) * HALF)
                        pav = ps_mm.tile([65, HALF], F32, tag="ps", name="ps")
                        for ktc in range(NTC):
                            nc.tensor.matmul(pav, mm(v_t[ktc][:, h_, :]),
                                             mm(e_t[ktc][:, sl]),
                                             start=(ktc == 0),
                                             stop=(ktc == NTC - 1))
                        pv_s = p_evt.tile([65, HALF], BF16, tag="pavs",
                                          name="pavs", bufs=6)
                        nc.vector.tensor_copy(out=pv_s, in_=pav)
                        pavs[th] = pv_s
                    e_heads[h_] = None
                    pavs_t[h_] = pavs

                def emit_norm(h_):
                    at = p_ao.tile([64, S], BF16, tag="ao", name="ao")
                    for th in range(2):
                        sl = slice(th * HALF, (th + 1) * HALF)
                        pv_s = pavs_t[h_][th]
                        rrb = p_rows.tile([1, HALF], MMF, tag="tmp2", name="rrb",
                                          bufs=4)
                        # 1/x via exp(-ln(x)): stays in the exp table set, so
                        # the softmax loop never swaps ACT tables.
                        lnd = p_rows.tile([1, HALF], F32, tag="tmp2",
                                          name="lnd", bufs=4)
                        nc.scalar.activation(out=lnd, in_=pv_s[64:65, :],
                                             func=AF.Ln)
                        nc.scalar.activation(out=rrb, in_=lnd, func=AF.Exp,
                                             bias=0.0, scale=-1.0)
                        prb = ps_mm.tile([64, HALF], F32, tag="ps", name="ps")
                        nc.tensor.matmul(prb, mm(ones[0:1, 0:64]), mm(rrb),
                                         start=True, stop=True)
                        rbs = p_evt.tile([64, HALF], BF16, tag="rbs",
                                         name="rbs")
                        nc.vector.tensor_copy(out=rbs, in_=prb)
                        nc.vector.tensor_tensor(out=at[:, sl],
                                                in0=pv_s[0:64, :], in1=rbs,
                                                op=OP.mult)
                    pavs_t[h_] = None
                    ao_t[h_] = at

                # pipeline: AV of head h rides inside the exp stream of head
                # h+1; score matmuls keep a two-head lead; the normalize
                # chain (which blocks on the Scalar exp batch) trails by one
                # head so it never stalls the PE score/AV streams.
                emit_scores(0)
                emit_scores(1)
                for h_ in range(H):
                    emit_av_mm(h_)
                    if h_ + 2 < H:
                        emit_scores(h_ + 2)
                    if h_ >= 1:
                        emit_norm(h_ - 1)
                emit_norm(H - 1)

                # o-projection + residual
                bto = load_bias("wo", NDC)
                for mc in range(NDC):
                    wo = p_wsm.tile([DH, H, 128], BF16, tag="wo", name="wo")
                    nc.sync.dma_start(out=wo, in_=dram["wo_w"][mc, :, :, :])
                    for th in range(2):
                        sl = slice(th * HALF, (th + 1) * HALF)
                        ps = ps_mm.tile([128, HALF], F32, tag="ps", name="ps")
                        for h_ in range(H):
                            nc.tensor.matmul(ps, mm(wo[:, h_, :]),
                                             mm(ao_t[h_][:, sl]),
                                             start=(h_ == 0),
                                             stop=(h_ == H - 1))
                        nc.vector.scalar_tensor_tensor(
                            out=x_t[mc][:, sl], in0=ps,
                            scalar=bto[:, mc:mc + 1], in1=x_t[mc][:, sl],
                            op0=OP.add, op1=OP.add)

        # ================= conv module =================
        # Tap schedule: a middle band of PE_TAPS runs as diag-matmul PSUM
        # accumulation; GPS_TAPS run as direct scalar_tensor_tensor on
        # GPSIMD; the rest are DVE scalar_tensor_tensor chains. All DVE tap
        # reads are kept 4B-aligned (even-j from hp_e, odd-j from a
        # one-element-shifted shadow hp_o made on the Scalar engine) so the
        # DVE runs in its 2x 16-bit mode. Emission is software-pipelined
        # per channel chunk so the PE never idles long enough to
        # re-throttle (HAM 3.4us window).
        def conv_module():
            xs = ln_stats_rows()
            NCC = CI // 128
            SHP = S + 2 * PAD
            n_pe = max(0, min(PE_TAPS, KCONV - 2))
            j0 = (KCONV - n_pe) // 2
            rest = [j for j in range(KCONV)
                    if not (j0 <= j < j0 + n_pe)]
            n_gps = max(0, min(GPS_TAPS, len(rest) - 1))
            gps_taps = rest[len(rest) - n_gps:] if n_gps else []
            dve_taps = rest[:len(rest) - n_gps]
            with ExitStack() as ph:
                p_hp = ph.enter_context(tc.tile_pool(name="p_hp", bufs=NCC))
                p_ca = ph.enter_context(tc.tile_pool(name="p_ca", bufs=NCC))
                p_acc = ph.enter_context(tc.tile_pool(name="p_acc", bufs=10))
                p_dg = ph.enter_context(tc.tile_pool(name="p_dg", bufs=4))
                p_dw = ph.enter_context(tc.tile_pool(name="p_dw", bufs=2))

                bt_a = load_bias("pw1", 2 * CI // 128)  # [128, 16]
                dwb = load_bias("dw", NCC)
                dww = p_dw.tile([128, NCC, KCONV], F32, tag="dww", name="dww")
                nc.sync.dma_start(out=dww, in_=dram["dw_w"].ap().rearrange(
                    "c p k -> p c k"))

                hp_e = [None] * NCC
                pe_ps = [None] * NCC
                acc_d = [None] * NCC
                acc_g = [None] * NCC
                ca_t = [None] * NCC

                def tap_src(pc, j):
                    return hp_e[pc][:, j:j + S]

                def st_pw1(pc):
                    """pw1 matmuls + sigmoid + GLU -> hp_e[pc]."""
                    hp_t = p_hp.tile([128, SHP], BF16, tag="hp", name="hp")
                    nc.vector.memset(hp_t[:, 0:PAD], 0.0)
                    nc.vector.memset(hp_t[:, PAD + S:], 0.0)
                    wt_a = p_w.tile([128, NDC, 128], MMF, tag="w", name="w")
                    nc.sync.dma_start(out=wt_a, in_=dram["pw1_w"][pc, :, :, :])
                    wt_g = p_w.tile([128, NDC, 128], MMF, tag="w", name="w")
                    nc.sync.dma_start(out=wt_g,
                                      in_=dram["pw1_w"][pc + NCC, :, :, :])
                    for th in range(2):
                        sl = slice(th * HALF, (th + 1) * HALF)
                        psa = ps_mm.tile([128, HALF], F32, tag="ps", name="ps")
                        psg = ps_mm.tile([128, HALF], F32, tag="ps", name="ps")
                        for kc in range(NDC):
                            nc.tensor.matmul(psg, mm(wt_g[:, kc, :]),
                                             mm(xs[kc][:, sl]),
                                             start=(kc == 0),
                                             stop=(kc == NDC - 1))
                        for kc in range(NDC):
                            nc.tensor.matmul(psa, mm(wt_a[:, kc, :]),
                                             mm(xs[kc][:, sl]),
                                             start=(kc == 0),
                                             stop=(kc == NDC - 1))
                        sig = p_evt.tile([128, HALF], BF16, tag="sig", name="sig")
                        nc.scalar.activation(out=sig, in_=psg, func=AF.Sigmoid,
                                             bias=bt_a[:, pc + NCC:pc + NCC + 1],
                                             scale=1.0)
                        abf = p_evt.tile([128, HALF], BF16, tag="sig",
                                         name="abf")
                        nc.scalar.activation(out=abf, in_=psa, func=AF.Identity,
                                             bias=bt_a[:, pc:pc + 1], scale=1.0)
                        # GLU product on GPSIMD: keeps the DVE free for taps
                        nc.gpsimd.tensor_tensor(
                            out=hp_t[:, PAD + th * HALF:PAD + (th + 1) * HALF],
                            in0=abf, in1=sig, op=OP.mult)
                    hp_e[pc] = hp_t

                def st_tap_pe(pc):
                    if not n_pe:
                        return
                    dgt = p_dg.tile([128, n_pe, 128], MMF, tag="dg", name="dg")
                    nc.sync.dma_start(out=dgt, in_=dram["dw_diag"][pc, :, :, :])
                    pps = []
                    for th in range(2):
                        sl0 = th * HALF
                        pp = ps_sc.tile([128, HALF], F32, tag="sc", name="sc")
                        for t in range(n_pe):
                            nc.tensor.matmul(
                                pp, mm(dgt[:, t, :]),
                                mm(hp_e[pc][:, j0 + t + sl0:
                                            j0 + t + sl0 + HALF]),
                                start=(t == 0), stop=(t == n_pe - 1))
                        pps.append(pp)
                    pe_ps[pc] = pps

                def st_tap_dve(pc):
                    acc = p_acc.tile([128, S], BF16, tag="acc", name="acc")
                    j_first = dve_taps[0]
                    nc.vector.tensor_scalar_mul(
                        out=acc, in0=tap_src(pc, j_first),
                        scalar1=dww[:, pc, j_first:j_first + 1])
                    for j in dve_taps[1:]:
                        nc.vector.scalar_tensor_tensor(
                            out=acc, in0=tap_src(pc, j),
                            scalar=dww[:, pc, j:j + 1], in1=acc,
                            op0=OP.mult, op1=OP.add)
                    acc_d[pc] = acc

                def st_tap_gps(pc):
                    """SG taps: the Scalar engine makes the per-tap products
                    (ACT Copy with per-partition scale — table-set free) and
                    GPSIMD chains the adds. Neither engine is near its
                    budget during conv, and it takes taps off the DVE/PE."""
                    if not gps_taps:
                        return
                    prods = []
                    for j in gps_taps:
                        pg = p_acc.tile([128, S], BF16, tag="acc", name="pg")
                        nc.scalar.mul(out=pg, in_=tap_src(pc, j),
                                      mul=dww[:, pc, j:j + 1])
                        prods.append(pg)
                    accg = prods[0]
                    for pg in prods[1:]:
                        nc.gpsimd.tensor_tensor(out=accg, in0=accg, in1=pg,
                                                op=OP.add)
                    acc_g[pc] = accg

                def st_merge(pc):
                    """Fold GPS acc into DVE acc (GPSIMD cannot touch PSUM),
                    then fold acc into the PE psum as an identity-matmul
                    accumulation (keeps the DVE out of 1x-mode PSUM ops)."""
                    if acc_g[pc] is not None:
                        nc.gpsimd.tensor_tensor(out=acc_d[pc], in0=acc_g[pc],
                                                in1=acc_d[pc], op=OP.add)
                    for th in range(2):
                        sl = slice(th * HALF, (th + 1) * HALF)
                        nc.tensor.matmul(pe_ps[pc][th], mm(ident),
                                         mm(acc_d[pc][:, sl]),
                                         start=False, stop=True)

                def st_silu(pc):
                    """silu(z) = z * sigmoid(z) via the sigmoid table (the
                    silu table lives in a different ACT set; using sigmoid
                    avoids a ~2.7us table swap per chunk). z and sigmoid(z)
                    read the psum on the Scalar engine; the product runs in
                    DVE 2x mode."""
                    ca = p_ca.tile([128, S], MMF, tag="ca", name="ca")
                    for th in range(2):
                        sl = slice(th * HALF, (th + 1) * HALF)
                        zt = p_evt.tile([128, HALF], BF16, tag="zsg",
                                        name="zt", bufs=4)
                        sg = p_evt.tile([128, HALF], BF16, tag="zsg",
                                        name="sg", bufs=4)
                        nc.scalar.activation(out=zt, in_=pe_ps[pc][th],
                                             func=AF.Identity,
                                             bias=dwb[:, pc:pc + 1], scale=1.0)
                        nc.scalar.activation(out=sg, in_=pe_ps[pc][th],
                                             func=AF.Sigmoid,
                                             bias=dwb[:, pc:pc + 1], scale=1.0)
                        nc.vector.tensor_tensor(out=ca[:, sl], in0=zt, in1=sg,
                                                op=OP.mult)
                    ca_t[pc] = ca

                # software-pipelined emission (2-chunk skew: pw1 + GLU of
                # chunk pc+2 overlap the GPS GLU hop and taps of chunk pc)
                st_pw1(0)
                st_pw1(1)
                for pc in range(NCC):
                    if pc + 2 < NCC:
                        st_pw1(pc + 2)
                    st_tap_pe(pc)
                    st_tap_dve(pc)
                    st_tap_gps(pc)
                    st_merge(pc)
                    if pc >= 1:
                        st_silu(pc - 1)
                st_silu(NCC - 1)

                bt2 = load_bias("pw2", NDC)

                def ev2(ps, mc, th, sl):
                    nc.vector.scalar_tensor_tensor(
                        out=x_t[mc][:, sl], in0=ps, scalar=bt2[:, mc:mc + 1],
                        in1=x_t[mc][:, sl], op0=OP.add, op1=OP.add)

                dense_mm("pw2", NCC, NDC, ca_t, ev2)

        # ================= run the block =================
        _mods = os.environ.get("CONF_MODULES", "ffacf")
        if "f" in _mods:
            with nc.named_scope("ff1") if SCOPES else nullcontext():
                ff_module("ff1a", "ff1b")
            shadow_cast()
            dbg_dump("ff1")
        print("built ff1", flush=True)
        if "a" in _mods:
            with nc.named_scope("attn") if SCOPES else nullcontext():
                attn_module()
            shadow_cast()
            dbg_dump("attn")
        print("built attn", flush=True)
        if "c" in _mods:
            with nc.named_scope("conv") if SCOPES else nullcontext():
                conv_module()
            shadow_cast()
            dbg_dump("conv")
        print("built conv", flush=True)
        if _mods.count("f") > 1:
            with nc.named_scope("ff2") if SCOPES else nullcontext():
                ff_module("ff2a", "ff2b")
            shadow_cast()
            dbg_dump("ff2")
        print("built ff2", flush=True)

        # final LN + transpose out (xs is already centered + scaled)
        if SCOPES:
            top.enter_context(nc.named_scope("final"))
        xs_f = ln_stats_rows()
        if not TRIV_FINAL:
            gt = load_bias("outln", NDC)
            bt = p_bias.tile([128, NDC], F32, tag="bias", name="bias")
            nc.sync.dma_start(out=bt, in_=dram["outln_b"].ap())
            for th in range(2):
                sl = slice(th * HALF, (th + 1) * HALF)
                for mc in range(NDC):
                    nc.vector.tensor_scalar(
                        out=xs_f[mc][:, sl], in0=xs_f[mc][:, sl],
                        scalar1=gt[:, mc:mc + 1], scalar2=bt[:, mc:mc + 1],
                        op0=OP.mult, op1=OP.add)
        for tck in range(NTC):
            pt = ps_mm.tile([128, D], MMF, tag="ps", name="ps")
            tsl = slice(tck * 128, (tck + 1) * 128)
            for mc in range(NDC):
                nc.tensor.transpose(out=pt[:, mc * 128:(mc + 1) * 128],
                                    in_=xs_f[mc][:, tsl], identity=ident)
            ob = p_evt.tile([128, D], F32, tag="ob", name="ob")
            nc.scalar.copy(out=ob, in_=pt)
            nc.sync.dma_start(out=y_d[tsl, :], in_=ob)

    _split_excess_waits(nc)
    return nc


def _split_excess_waits(nc, limit=1):
    """This walrus build caps sync-waits per instruction very low; hoist
    excess waits onto single-wait NOPs inserted before the instruction on
    the same engine (same-engine program order preserves the guarantee)."""
    from concourse import mybir
    cnt = 0
    for fn in nc.m.functions:
        for bb in fn.blocks:
            out = []
            for ins in bb.instructions:
                si = getattr(ins, "sync_info", None)
                if si is not None and si.on_wait and len(si.on_wait) > limit:
                    waits = list(si.on_wait)
                    keep = waits[:limit]
                    for w in waits[limit:]:
                        cnt += 1
                        out.append(mybir.InstNoOp(
                            name=f"waitnop_{cnt}",
                            engine=ins.engine,
                            sync_info=mybir.SyncInfo(on_wait=[w],
                                                     on_update=[]),
                        ))
                    si.on_wait = keep
                out.append(ins)
            bb.instructions = out
    return cnt


_CACHE = {}


def _get_program(has_vb, triv_final):
    key = (MM_MODE, GPS_TAPS, PE_TAPS, DBG, has_vb, triv_final)
    if key not in _CACHE:
        build_program._has_vb = has_vb
        build_program._triv_final = triv_final
        _CACHE[key] = build_program()
    return _CACHE[key]


LAST_EXEC_NS = None


def kernel(**inputs):
    global LAST_EXEC_NS
    from concourse.bass_utils import run_bass_kernel_spmd

    w = prep_inputs(inputs)
    has_vb = w.pop("_has_vb")
    triv_final = w.pop("_triv_final")
    nc = _get_program(has_vb, triv_final)

    mdt = _np_mm_dtype()
    x = np.asarray(inputs["x"], np.float32)
    in_maps = []
    for c in range(NCORES):
        m = dict(w)
        m["x"] = np.ascontiguousarray(x[c].T).astype(mdt)
        in_maps.append(m)
    trace = os.environ.get("CONF_TRACE", "0") == "1"
    res = run_bass_kernel_spmd(nc, in_maps, core_ids=list(range(NCORES)),
                               trace=trace)
    LAST_EXEC_NS = res.exec_time_ns
    out = np.stack([res.results[c]["y"] for c in range(NCORES)], 0)
    return out.astype(np.float32)



# revision 33
# speedup vs baseline: 1.3332x; 1.3332x over previous
"""Trainium2 Bass kernel for a Conformer block (B=8, S=1024, D=512).

Sharding: data-parallel over batch — 1 batch element per NeuronCore, 8 cores,
no collectives.

Per-core layout strategy: the residual stream lives in SBUF feature-major
([D, S]) in bf16; every linear layer is then a natural PE matmul with the
stored [in, out] weight as lhsT, all in bf16 (f32 PSUM accumulation) — bf16
keeps the PE out of the fp32-HIGH power-throttle regime. LayerNorm
gains/biases, the attention scale, and BatchNorm are folded into the weights
on the host; x arrives host-transposed ([D, S]) and pre-cast to bf16.
LayerNorm mean/var come from ones-vector matmuls on the PE; mean and rstd
are applied to the matmul input via two K=1 broadcast matmuls plus two
elementwise ops (no augmented-row matmuls); rstd/softmax reciprocals use the
Act-engine Rsqrt/Reciprocal tables (tolerance is loose). Softmax
denominators come for free from a ones column appended to V. The depthwise
conv splits its 31 taps three ways: a middle band as diag-matmul PSUM
accumulation on the otherwise-idle PE, a GPSIMD tail fed by DVE products,
and the rest as DVE scalar_tensor_tensor chains; the conv module is phased
(all GLU sigmoids -> taps -> all SiLUs) to avoid Act table thrash.
"""

import os
import numpy as np

# ---------------- problem constants (hardcoded) ----------------
B, S, D = 8, 1024, 512
H, DH = 8, 64
FFI, CI, KCONV = 1024, 1024, 31
EPS = 1e-5
NCORES = 8
PAD = (KCONV - 1) // 2  # 15
NDC = D // 128    # 4  d-chunks
NTC = S // 128    # 8  t-chunks
HALF = S // 2     # 512

MM_MODE = os.environ.get("CONF_MM_MODE", "bf16")  # bf16 | f32r | f32
SCOPES = os.environ.get("CONF_SCOPES", "1") == "1"
GPS_TAPS = int(os.environ.get("CONF_GPS_TAPS", "0"))  # taps on gpsimd (0:
# the Pool engine rejects AP-scalar ops on this target, so direct GPS taps
# cannot run; the hook remains for a product-fed variant)
PE_TAPS = int(os.environ.get("CONF_PE_TAPS", "20"))  # taps as PE diag matmuls
DBG = os.environ.get("CONF_DEBUG_STAGES", "0") == "1"


# ---------------- tile-framework workaround ----------------
def _patch_tile_drain():
    """This walrus build rejects >1 sync-wait on TPB_CTRL (Drain/NOP)
    instructions; spread the TileContext tail-drain waits across
    single-wait NOPs."""
    import concourse.tile as tile
    from concourse.vector_clock import ScopedClock
    from concourse import mybir

    if getattr(tile.TileContext, "_drain_patched", False):
        return

    def _drain_and_barrier(self, tick_clock, wait_clock):
        nc = self.nc
        carrier = nc.sync.nop(nofuse=True, hint="tail_wait_carrier")
        wait_clock.add_sem_waits(
            carrier.ins, ScopedClock({None: tick_clock.global_clock})
        )
        waits = list(carrier.ins.sync_info.on_wait)
        if len(waits) > 1:
            carrier.ins.sync_info.on_wait = waits[:1]
            for w in waits[1:]:
                nxt = nc.sync.nop(nofuse=True, hint="tail_wait_carrier")
                nxt.ins.sync_info = mybir.SyncInfo(on_wait=[w], on_update=[])
        nc.sync.drain()
        nc.all_engine_barrier()
        assert self.sems is not None
        popped = nc._tile_sem_poison_stack.pop()
        assert popped is self._sem_poison
        nc.clear_and_free_semaphores(list(self.sems.allocated().values()))
        nc.all_engine_barrier()

    tile.TileContext._drain_and_barrier = _drain_and_barrier
    tile.TileContext._drain_patched = True


def _np_mm_dtype():
    import ml_dtypes
    return ml_dtypes.bfloat16 if MM_MODE == "bf16" else np.float32


# ---------------- host-side weight preparation ----------------
def _blob_lhsT(w, nk, nm):
    """[K, M] -> [Mc, 128(p), nk, 128(m)] contiguous (lhsT tile layout)."""
    K, M = w.shape
    assert K == nk * 128 and M == nm * 128
    return np.ascontiguousarray(
        w.reshape(nk, 128, nm, 128).transpose(2, 1, 0, 3)
    ).astype(_np_mm_dtype())


def _blob_bias(b, nm):
    """[M] -> [128, Mc] (per-partition bias columns)."""
    return np.ascontiguousarray(b.reshape(nm, 128).T).astype(np.float32)


def prep_inputs(inp):
    """Fold LN gains/biases, attention scale, BatchNorm, and FF 0.5 scales
    into weights. Returns dict of DRAM arrays shared by all cores."""
    f64 = lambda a: np.asarray(a, np.float64)
    mdt = _np_mm_dtype()
    out = {}

    def ln_matmul_group(pfx, g, lb, w, wb, nm, scale=1.0):
        wg = f64(w) * f64(g)[:, None] * scale
        out[f"{pfx}_w"] = _blob_lhsT(wg, NDC, nm)
        bias = (f64(wb) + f64(lb) @ f64(w)) * scale
        out[f"{pfx}_b"] = _blob_bias(bias, nm)

    # FF1
    ln_matmul_group("ff1a", inp["ff1_ln_g"], inp["ff1_ln_b"],
                    inp["ff1_w1"], inp["ff1_b1"], FFI // 128)
    out["ff1b_w"] = _blob_lhsT(f64(inp["ff1_w2"]) * 0.5, FFI // 128, NDC)
    out["ff1b_b"] = _blob_bias(f64(inp["ff1_b2"]) * 0.5, NDC)

    # attention
    ln_matmul_group("wq", inp["attn_ln_g"], inp["attn_ln_b"],
                    inp["q_w"], inp["q_b"], NDC, scale=DH ** -0.5)
    ln_matmul_group("wk", inp["attn_ln_g"], inp["attn_ln_b"],
                    inp["k_w"], inp["k_b"], NDC)
    # v: rhs layout [kc, p, n]
    wvg = f64(inp["v_w"]) * f64(inp["attn_ln_g"])[:, None]
    out["wv_w"] = np.ascontiguousarray(
        wvg.reshape(NDC, 128, H * DH)
    ).astype(mdt)
    vb = f64(inp["v_b"]) + f64(inp["attn_ln_b"]) @ f64(inp["v_w"])
    out["wv_bias"] = vb.astype(mdt).reshape(1, H * DH)
    out["_has_vb"] = bool(np.abs(vb).max() > 0)
    # o: [Mc, 64(p), H, 128(m)]
    import ml_dtypes
    out["wo_w"] = np.ascontiguousarray(
        f64(inp["o_w"]).reshape(H, DH, NDC, 128).transpose(2, 1, 0, 3)
    ).astype(ml_dtypes.bfloat16)
    out["wo_b"] = _blob_bias(f64(inp["o_b"]), NDC)

    # conv module
    ln_matmul_group("pw1", inp["conv_ln_g"], inp["conv_ln_b"],
                    inp["pw1_w"], inp["pw1_b"], 2 * CI // 128)
    inv = f64(inp["bn_g"]) / np.sqrt(f64(inp["bn_var"]) + EPS)
    dwf = f64(inp["dw_w"])[:, 0, :] * inv[:, None]  # [CI, K]
    out["dw_w"] = np.ascontiguousarray(
        dwf.reshape(CI // 128, 128, KCONV)
    ).astype(np.float32)
    cb = (f64(inp["dw_b"]) - f64(inp["bn_mean"])) * inv + f64(inp["bn_b"])
    out["dw_b"] = _blob_bias(cb, CI // 128)
    out["pw2_w"] = _blob_lhsT(f64(inp["pw2_w"]), CI // 128, NDC)
    out["pw2_b"] = _blob_bias(f64(inp["pw2_b"]), NDC)
    # diag(w[c,j]) lhsT blobs for the PE-matmul taps: [NCC, K, 128, 128]
    n_pe = max(0, min(PE_TAPS, KCONV))
    if n_pe:
        dg = np.zeros((CI // 128, n_pe, 128, 128), np.float64)
        dwr = dwf.reshape(CI // 128, 128, KCONV)
        j0 = (KCONV - n_pe) // 2  # PE takes a middle band of taps
        idx = np.arange(128)
        for pc in range(CI // 128):
            for t in range(n_pe):
                dg[pc, t, idx, idx] = dwr[pc, :, j0 + t]
        out["dw_diag"] = dg.astype(mdt)

    # FF2
    ln_matmul_group("ff2a", inp["ff2_ln_g"], inp["ff2_ln_b"],
                    inp["ff2_w1"], inp["ff2_b1"], FFI // 128)
    out["ff2b_w"] = _blob_lhsT(f64(inp["ff2_w2"]) * 0.5, FFI // 128, NDC)
    out["ff2b_b"] = _blob_bias(f64(inp["ff2_b2"]) * 0.5, NDC)

    # final LN
    out["outln_g"] = _blob_bias(f64(inp["out_ln_g"]), NDC)
    out["outln_b"] = _blob_bias(f64(inp["out_ln_b"]), NDC)
    out["_triv_final"] = bool(
        np.allclose(inp["out_ln_g"], 1.0) and np.allclose(inp["out_ln_b"], 0.0)
    )
    out["ident"] = np.eye(128, dtype=mdt)
    out["ones_c"] = np.ones((128, 128), dtype=mdt)
    return out


# ---------------- kernel builder ----------------
def build_program():
    _patch_tile_drain()
    import concourse.bass as bass
    import concourse.tile as tile
    from concourse import mybir
    from contextlib import ExitStack, nullcontext

    dt = mybir.dt
    AF = mybir.ActivationFunctionType
    OP = mybir.AluOpType
    F32 = dt.float32
    BF16 = dt.bfloat16
    F8 = dt.float8e4
    DR = mybir.MatmulPerfMode.DoubleRow

    MMF = {"bf16": dt.bfloat16, "f32r": dt.float32r,
           "f32": dt.float32}[MM_MODE]

    def mm(ap):
        return ap

    nc = bass.Bass("TRN2", target_bir_lowering=False, debug=False)

    def act_unsafe(out, in_, func, bias=0.0, scale=1.0):
        """Emit an InstActivation bypassing the Reciprocal/Rsqrt accuracy
        guard in bass (tolerance here is 2e-2; the table approximation is
        fine and ~5x faster than the DVE multi-pass reciprocal)."""
        eng = nc.scalar
        inputs = [eng.lower_ap(in_)]
        for arg in (bias, scale, 0.0):
            if isinstance(arg, bass.AP):
                inputs.append(eng.lower_ap(arg))
            else:
                inputs.append(mybir.ImmediateValue(dtype=mybir.dt.float32,
                                                   value=float(arg)))
        return eng.add_instruction(mybir.InstActivation(
            name=nc.get_next_instruction_name(),
            func=func,
            ins=inputs,
            outs=[eng.lower_ap(out)],
        ))

    # ---- DRAM declarations ----
    x_d = nc.dram_tensor("x", [D, S], MMF, kind="ExternalInput")
    y_d = nc.dram_tensor("y", [S, D], F32, kind="ExternalOutput")
    dram = {}

    def din(name, shape, dtp=None):
        dram[name] = nc.dram_tensor(name, list(shape), dtp or F32,
                                    kind="ExternalInput")
        return dram[name]

    def dinm(name, shape):
        return din(name, shape, MMF)

    dinm("ident", [128, 128])
    dinm("ones_c", [128, 128])
    for pfx, nm in [("ff1a", FFI // 128), ("wq", NDC), ("wk", NDC),
                    ("pw1", 2 * CI // 128), ("ff2a", FFI // 128)]:
        dinm(f"{pfx}_w", [nm, 128, NDC, 128])
        din(f"{pfx}_b", [128, nm])
    for pfx, nk, nm in [("ff1b", FFI // 128, NDC), ("pw2", CI // 128, NDC),
                        ("ff2b", FFI // 128, NDC)]:
        dinm(f"{pfx}_w", [nm, 128, nk, 128])
        din(f"{pfx}_b", [128, nm])
    dinm("wv_w", [NDC, 128, H * DH])
    dinm("wv_bias", [1, H * DH])
    N_PE_TAPS = max(0, min(PE_TAPS, KCONV))
    if N_PE_TAPS:
        dinm("dw_diag", [CI // 128, N_PE_TAPS, 128, 128])
    din("wo_w", [NDC, DH, H, 128], BF16)
    din("wo_b", [128, NDC])
    din("dw_w", [CI // 128, 128, KCONV])
    din("dw_b", [128, CI // 128])
    din("outln_g", [128, NDC])
    din("outln_b", [128, NDC])

    dbg_d = {}
    if DBG:
        for s_ in ["ff1", "attn", "conv", "ff2"]:
            dbg_d[s_] = nc.dram_tensor(f"dbg_{s_}", [D, S], MMF,
                                       kind="ExternalOutput")

    HAS_VB = build_program._has_vb
    TRIV_FINAL = build_program._triv_final

    with tile.TileContext(nc) as tc, ExitStack() as top:
        top.enter_context(nc.allow_low_precision(
            reason="bf16 compute is intentional"))
        # ---- global pools ----
        p_x = top.enter_context(tc.tile_pool(name="p_x", bufs=1))
        p_const = top.enter_context(tc.tile_pool(name="p_const", bufs=1))
        p_rows = top.enter_context(tc.tile_pool(name="p_rows", bufs=2))
        p_sq = top.enter_context(tc.tile_pool(name="p_sq", bufs=2))
        p_w = top.enter_context(tc.tile_pool(name="p_w", bufs=3))
        p_wsm = top.enter_context(tc.tile_pool(name="p_wsm", bufs=2))
        p_bias = top.enter_context(tc.tile_pool(name="p_bias", bufs=2))
        p_xs = top.enter_context(tc.tile_pool(name="p_xs", bufs=4))
        p_evt = top.enter_context(tc.tile_pool(name="p_evt", bufs=2))
        # PSUM budget is 8 banks. ps_sc feeds the two MM->ACT ping-pong
        # streams (attention scores->exp, conv tap psums) with enough slots
        # that the producer never waits on the consumer's semaphore
        # round-trip; ps_mm covers everything else; ps_st holds one LN stat.
        ps_mm = top.enter_context(tc.tile_pool(name="ps_mm", bufs=3, space="PSUM"))
        ps_sc = top.enter_context(tc.tile_pool(name="ps_sc", bufs=4, space="PSUM"))
        ps_st = top.enter_context(tc.tile_pool(name="ps_st", bufs=1, space="PSUM"))

        ident = p_const.tile([128, 128], MMF, tag="ident", name="ident")
        nc.sync.dma_start(out=ident, in_=dram["ident"].ap())
        ones = p_const.tile([128, 128], MMF, tag="ones", name="ones")
        nc.sync.dma_start(out=ones, in_=dram["ones_c"].ap())
        epst = p_const.tile([128, 1], F32, tag="epst", name="epst")
        nc.vector.memset(epst, EPS)
        ones8 = p_const.tile([128, 2, 64], F8, tag="ones8", name="ones8")
        nc.vector.memset(ones8, 1.0)

        # ---- load x (already feature-major + cast on host) ----
        x_t = [p_x.tile([128, S], MMF, tag=f"x{i}", name=f"x{i}") for i in range(NDC)]
        xb_t = x_t
        for mc in range(NDC):
            nc.sync.dma_start(out=x_t[mc], in_=x_d[mc * 128:(mc + 1) * 128, :])

        def shadow_cast():
            pass

        # ---- helpers ----
        def ln_stats_rows():
            """LN over feature axis of x_t. Returns xs tiles with
            xs = (x - mean) * rstd (feature-major, centered + scaled)."""
            rstd_b = p_rows.tile([1, S], MMF, tag="rstdb", name="rstdb")
            mrow = p_rows.tile([1, S], MMF, tag="mrow", name="mrow")
            c1 = 1.0 / D
            for th in range(2):
                sl = slice(th * HALF, (th + 1) * HALF)
                s1 = ps_st.tile([1, HALF], F32, tag="st", name="st")
                s2 = ps_mm.tile([1, HALF], F32, tag="ps", name="st2")
                for kc in range(NDC):
                    nc.tensor.matmul(s1, mm(ones[:, 0:1]),
                                     mm(xb_t[kc][:, sl]),
                                     start=(kc == 0), stop=(kc == NDC - 1))
                for kc in range(NDC):
                    sq = p_sq.tile([128, HALF], MMF, tag="sq", name="sq")
                    nc.scalar.square(out=sq, in_=xb_t[kc][:, sl])
                    nc.tensor.matmul(s2, mm(ones[:, 0:1]), mm(sq),
                                     start=(kc == 0), stop=(kc == NDC - 1))
                mean_s = p_rows.tile([1, HALF], F32, tag="tmp",
                                     name="mean_s", bufs=4)
                nc.vector.tensor_scalar_mul(out=mean_s, in0=s1, scalar1=c1)
                msq = p_rows.tile([1, HALF], F32, tag="tmp", name="msq", bufs=4)
                nc.vector.tensor_tensor(out=msq, in0=mean_s, in1=mean_s,
                                        op=OP.mult)
                vpe = p_rows.tile([1, HALF], F32, tag="tmp", name="vpe", bufs=4)
                nc.vector.scalar_tensor_tensor(
                    out=vpe, in0=s2, scalar=c1, in1=msq,
                    op0=OP.mult, op1=OP.subtract)
                # rsqrt via exp(-0.5*ln(v+eps)): ln+exp share one ACT table
                # set (natural_log_exp_and_others), so no table swap against
                # the attention exp / softmax reciprocal path.
                lnv = p_rows.tile([1, HALF], F32, tag="tmp", name="lnv",
                                  bufs=4)
                nc.scalar.activation(out=lnv, in_=vpe, func=AF.Ln,
                                     bias=epst[0:1, :], scale=1.0)
                nc.scalar.activation(out=rstd_b[:, sl], in_=lnv, func=AF.Exp,
                                     bias=0.0, scale=-0.5)
                nc.vector.scalar_tensor_tensor(
                    out=mrow[:, sl], in0=mean_s, scalar=-1.0,
                    in1=rstd_b[:, sl], op0=OP.mult, op1=OP.mult)
            # xs = x * bcast(rstd) + bcast(-mean*rstd); the broadcast tiles
            # are copied to SBUF once (GPSIMD cannot read PSUM) and the
            # per-chunk applies are split DVE/GPSIMD to keep the DVE free.
            xs = [p_xs.tile([128, S], MMF, tag="xs", name="xs") for _ in range(NDC)]
            for th in range(2):
                sl = slice(th * HALF, (th + 1) * HALF)
                pb = ps_mm.tile([128, HALF], F32, tag="ps", name="ps")
                nc.tensor.matmul(pb, mm(ones[0:1, :]), mm(rstd_b[:, sl]),
                                 start=True, stop=True)
                pm = ps_mm.tile([128, HALF], F32, tag="ps", name="ps")
                nc.tensor.matmul(pm, mm(ones[0:1, :]), mm(mrow[:, sl]),
                                 start=True, stop=True)
                pbs = p_evt.tile([128, HALF], BF16, tag="zsg", name="pbs",
                                 bufs=4)
                pms = p_evt.tile([128, HALF], BF16, tag="zsg", name="pms",
                                 bufs=4)
                nc.scalar.copy(out=pbs, in_=pb)
                nc.scalar.copy(out=pms, in_=pm)
                for kc in range(NDC):
                    eng = nc.vector if kc < 2 else nc.gpsimd
                    eng.tensor_tensor(out=xs[kc][:, sl],
                                      in0=x_t[kc][:, sl], in1=pbs,
                                      op=OP.mult)
                    eng.tensor_tensor(out=xs[kc][:, sl],
                                      in0=xs[kc][:, sl], in1=pms,
                                      op=OP.add)
            return xs

        def load_bias(pfx, nm):
            bt = p_bias.tile([128, nm], F32, tag="bias", name="bias")
            nc.sync.dma_start(out=bt, in_=dram[f"{pfx}_b"].ap())
            return bt

        def dense_mm(pfx, nk, nm, rhs_tiles, evict):
            """plain contraction over nk chunks of rhs_tiles."""
            for mc in range(nm):
                wt = p_w.tile([128, nk, 128], MMF, tag="w", name="w")
                nc.sync.dma_start(out=wt, in_=dram[f"{pfx}_w"][mc, :, :, :])
                for th in range(2):
                    sl = slice(th * HALF, (th + 1) * HALF)
                    ps = ps_mm.tile([128, HALF], F32, tag="ps", name="ps")
                    for kc in range(nk):
                        nc.tensor.matmul(ps, mm(wt[:, kc, :]),
                                         mm(rhs_tiles[kc][:, sl]),
                                         start=(kc == 0), stop=(kc == nk - 1))
                    evict(ps, mc, th, sl)

        def dbg_dump(name):
            if DBG:
                for mc in range(NDC):
                    nc.sync.dma_start(
                        out=dbg_d[name][mc * 128:(mc + 1) * 128, :],
                        in_=x_t[mc])

        # ================= FF module =================
        def ff_module(pa, pb):
            xs = ln_stats_rows()
            with tc.tile_pool(name="p_h", bufs=FFI // 128) as p_h:
                bt1 = load_bias(pa, FFI // 128)
                h = [p_h.tile([128, S], MMF, tag="h", name="h") for _ in range(FFI // 128)]

                def ev1(ps, mc, th, sl):
                    nc.scalar.activation(out=h[mc][:, sl], in_=ps, func=AF.Silu,
                                         bias=bt1[:, mc:mc + 1], scale=1.0)

                dense_mm(pa, NDC, FFI // 128, xs, ev1)
                bt2 = load_bias(pb, NDC)

                def ev2(ps, mc, th, sl):
                    nc.vector.scalar_tensor_tensor(
                        out=x_t[mc][:, sl], in0=ps, scalar=bt2[:, mc:mc + 1],
                        in1=x_t[mc][:, sl], op0=OP.add, op1=OP.add)

                dense_mm(pb, FFI // 128, NDC, h, ev2)

        # ================= attention =================
        def attn_module():
            xs = ln_stats_rows()
            with ExitStack() as ph:
                p_qk = ph.enter_context(tc.tile_pool(name="p_qk", bufs=8))
                p_v = ph.enter_context(tc.tile_pool(name="p_v", bufs=NTC))
                p_exp = ph.enter_context(tc.tile_pool(name="p_exp", bufs=24))
                p_ao = ph.enter_context(tc.tile_pool(name="p_ao", bufs=H))
                p_wv = ph.enter_context(tc.tile_pool(name="p_wv", bufs=1))

                q_t = [p_qk.tile([128, S], BF16, tag="qk", name="qk") for _ in range(NDC)]
                k_t = [p_qk.tile([128, S], BF16, tag="qk", name="qk") for _ in range(NDC)]
                btq = load_bias("wq", NDC)
                btk = load_bias("wk", NDC)

                def evq(ps, mc, th, sl):
                    nc.vector.tensor_scalar_add(out=q_t[mc][:, sl], in0=ps,
                                                scalar1=btq[:, mc:mc + 1])

                def evk(ps, mc, th, sl):
                    nc.vector.tensor_scalar_add(out=k_t[mc][:, sl], in0=ps,
                                                scalar1=btk[:, mc:mc + 1])

                dense_mm("wq", NDC, NDC, xs, evq)
                dense_mm("wk", NDC, NDC, xs, evk)

                # v (token-major, with ones column per head)
                wv = p_wv.tile([128, NDC, H * DH], MMF, tag="wv", name="wv")
                nc.sync.dma_start(out=wv, in_=dram["wv_w"].ap().rearrange(
                    "k p n -> p k n"))
                if HAS_VB:
                    wvb = p_wv.tile([1, H * DH], MMF, tag="wvb", name="wvb")
                    nc.sync.dma_start(out=wvb, in_=dram["wv_bias"].ap())
                # v (token-major, with ones column per head)
                v_t = []
                for tck in range(NTC):
                    vt = p_v.tile([128, H, DH + 1], BF16, tag="v", name="v")
                    nc.vector.memset(vt[:, :, DH:DH + 1], 1.0)
                    pv = ps_mm.tile([128, H * DH], F32, tag="ps", name="ps")
                    tsl = slice(tck * 128, (tck + 1) * 128)
                    for kc in range(NDC):
                        nc.tensor.matmul(pv, mm(xs[kc][:, tsl]),
                                         mm(wv[:, kc, :]),
                                         start=(kc == 0),
                                         stop=(kc == NDC - 1 and not HAS_VB))
                    if HAS_VB:
                        nc.tensor.matmul(pv, mm(ones[0:1, :]), mm(wvb),
                                         start=False, stop=True)
                    nc.vector.tensor_copy(
                        out=vt[:, :, 0:DH],
                        in_=pv.rearrange("p (h d) -> p h d", h=H))
                    v_t.append(vt)

                # scores -> exp -> AV -> normalize, software-pipelined with a
                # one-head skew: the PE runs head h+1's score matmuls while
                # the Scalar engine exps head h, so neither engine waits and
                # the PE never idles past the HAM re-throttle window.
                ao_t = [None] * H
                e_heads = [None] * H

                def emit_scores(h_):
                    hp, sub = h_ // 2, h_ % 2
                    base = sub * 64
                    e_t = []
                    for ktc in range(NTC):
                        et = p_exp.tile([128, S], BF16, tag="exp", name="exp")
                        ksl = slice(ktc * 128, (ktc + 1) * 128)
                        for th in range(2):
                            sl = slice(th * HALF, (th + 1) * HALF)
                            pss = ps_sc.tile([128, HALF], F32, tag="sc",
                                             name="sc")
                            nc.tensor.matmul(
                                pss,
                                mm(k_t[hp][base:base + 64, ksl]),
                                mm(q_t[hp][base:base + 64, sl]),
                                start=True, stop=True,
                                tile_position=(base, 0))
                            nc.scalar.activation(out=et[:, sl], in_=pss,
                                                 func=AF.Exp)
                        e_t.append(et)
                    e_heads[h_] = e_t

                pavs_t = [None] * H

                def emit_av_mm(h_):
                    """AV matmuls; pav is copied to SBUF immediately so the
                    psum bank frees within ~0.5us."""
                    e_t = e_heads[h_]
                    pavs = [None, None]
                    for th in range(2):
                        sl = slice(th * HALF, (th + 1) * HALF)
                        pav = ps_mm.tile([65, HALF], F32, tag="ps", name="ps")
                        for ktc in range(NTC):
                            nc.tensor.matmul(pav, mm(v_t[ktc][:, h_, :]),
                                             mm(e_t[ktc][:, sl]),
                                             start=(ktc == 0),
                                             stop=(ktc == NTC - 1))
                        pv_s = p_evt.tile([65, HALF], BF16, tag="pavs",
                                          name="pavs", bufs=6)
                        nc.vector.tensor_copy(out=pv_s, in_=pav)
                        pavs[th] = pv_s
                    e_heads[h_] = None
                    pavs_t[h_] = pavs

                def emit_norm(h_):
                    at = p_ao.tile([64, S], BF16, tag="ao", name="ao")
                    for th in range(2):
                        sl = slice(th * HALF, (th + 1) * HALF)
                        pv_s = pavs_t[h_][th]
                        rrb = p_rows.tile([1, HALF], MMF, tag="tmp2", name="rrb",
                                          bufs=4)
                        # 1/x via exp(-ln(x)): stays in the exp table set, so
                        # the softmax loop never swaps ACT tables.
                        lnd = p_rows.tile([1, HALF], F32, tag="tmp2",
                                          name="lnd", bufs=4)
                        nc.scalar.activation(out=lnd, in_=pv_s[64:65, :],
                                             func=AF.Ln)
                        nc.scalar.activation(out=rrb, in_=lnd, func=AF.Exp,
                                             bias=0.0, scale=-1.0)
                        prb = ps_mm.tile([64, HALF], F32, tag="ps", name="ps")
                        nc.tensor.matmul(prb, mm(ones[0:1, 0:64]), mm(rrb),
                                         start=True, stop=True)
                        rbs = p_evt.tile([64, HALF], BF16, tag="rbs",
                                         name="rbs")
                        nc.vector.tensor_copy(out=rbs, in_=prb)
                        nc.vector.tensor_tensor(out=at[:, sl],
                                                in0=pv_s[0:64, :], in1=rbs,
                                                op=OP.mult)
                    pavs_t[h_] = None
                    ao_t[h_] = at

                # pipeline: AV of head h rides inside the exp stream of head
                # h+1; score matmuls keep a two-head lead; the normalize
                # chain (which blocks on the Scalar exp batch) trails by one
                # head so it never stalls the PE score/AV streams.
                emit_scores(0)
                emit_scores(1)
                for h_ in range(H):
                    emit_av_mm(h_)
                    if h_ + 2 < H:
                        emit_scores(h_ + 2)
                    if h_ >= 1:
                        emit_norm(h_ - 1)
                emit_norm(H - 1)

                # o-projection + residual
                bto = load_bias("wo", NDC)
                for mc in range(NDC):
                    wo = p_wsm.tile([DH, H, 128], BF16, tag="wo", name="wo")
                    nc.sync.dma_start(out=wo, in_=dram["wo_w"][mc, :, :, :])
                    for th in range(2):
                        sl = slice(th * HALF, (th + 1) * HALF)
                        ps = ps_mm.tile([128, HALF], F32, tag="ps", name="ps")
                        for h_ in range(H):
                            nc.tensor.matmul(ps, mm(wo[:, h_, :]),
                                             mm(ao_t[h_][:, sl]),
                                             start=(h_ == 0),
                                             stop=(h_ == H - 1))
                        nc.vector.scalar_tensor_tensor(
                            out=x_t[mc][:, sl], in0=ps,
                            scalar=bto[:, mc:mc + 1], in1=x_t[mc][:, sl],
                            op0=OP.add, op1=OP.add)

        # ================= conv module =================
        # Tap schedule: a middle band of PE_TAPS runs as diag-matmul PSUM
        # accumulation; GPS_TAPS run as direct scalar_tensor_tensor on
        # GPSIMD; the rest are DVE scalar_tensor_tensor chains. All DVE tap
        # reads are kept 4B-aligned (even-j from hp_e, odd-j from a
        # one-element-shifted shadow hp_o made on the Scalar engine) so the
        # DVE runs in its 2x 16-bit mode. Emission is software-pipelined
        # per channel chunk so the PE never idles long enough to
        # re-throttle (HAM 3.4us window).
        def conv_module():
            xs = ln_stats_rows()
            NCC = CI // 128
            SHP = S + 2 * PAD
            n_pe = max(0, min(PE_TAPS, KCONV - 2))
            j0 = (KCONV - n_pe) // 2
            rest = [j for j in range(KCONV)
                    if not (j0 <= j < j0 + n_pe)]
            n_gps = max(0, min(GPS_TAPS, len(rest) - 1))
            gps_taps = rest[len(rest) - n_gps:] if n_gps else []
            dve_taps = rest[:len(rest) - n_gps]
            with ExitStack() as ph:
                p_hp = ph.enter_context(tc.tile_pool(name="p_hp", bufs=NCC))
                p_ca = ph.enter_context(tc.tile_pool(name="p_ca", bufs=NCC))
                p_acc = ph.enter_context(tc.tile_pool(name="p_acc",
                         bufs=4 if not GPS_TAPS else 2 * (GPS_TAPS + 1)))
                p_dg = ph.enter_context(tc.tile_pool(name="p_dg", bufs=4))
                p_dw = ph.enter_context(tc.tile_pool(name="p_dw", bufs=2))

                bt_a = load_bias("pw1", 2 * CI // 128)  # [128, 16]
                dwb = load_bias("dw", NCC)
                dww = p_dw.tile([128, NCC, KCONV], F32, tag="dww", name="dww")
                nc.sync.dma_start(out=dww, in_=dram["dw_w"].ap().rearrange(
                    "c p k -> p c k"))

                hp_e = [None] * NCC
                pe_ps = [None] * NCC
                acc_d = [None] * NCC
                acc_g = [None] * NCC
                ca_t = [None] * NCC

                def tap_src(pc, j):
                    return hp_e[pc][:, j:j + S]

                def st_pw1(pc):
                    """pw1 matmuls + sigmoid + GLU -> hp_e[pc]."""
                    hp_t = p_hp.tile([128, SHP], BF16, tag="hp", name="hp")
                    nc.vector.memset(hp_t[:, 0:PAD], 0.0)
                    nc.vector.memset(hp_t[:, PAD + S:], 0.0)
                    wt_a = p_w.tile([128, NDC, 128], MMF, tag="w", name="w")
                    nc.sync.dma_start(out=wt_a, in_=dram["pw1_w"][pc, :, :, :])
                    wt_g = p_w.tile([128, NDC, 128], MMF, tag="w", name="w")
                    nc.sync.dma_start(out=wt_g,
                                      in_=dram["pw1_w"][pc + NCC, :, :, :])
                    for th in range(2):
                        sl = slice(th * HALF, (th + 1) * HALF)
                        psa = ps_mm.tile([128, HALF], F32, tag="ps", name="ps")
                        psg = ps_mm.tile([128, HALF], F32, tag="ps", name="ps")
                        for kc in range(NDC):
                            nc.tensor.matmul(psg, mm(wt_g[:, kc, :]),
                                             mm(xs[kc][:, sl]),
                                             start=(kc == 0),
                                             stop=(kc == NDC - 1))
                        for kc in range(NDC):
                            nc.tensor.matmul(psa, mm(wt_a[:, kc, :]),
                                             mm(xs[kc][:, sl]),
                                             start=(kc == 0),
                                             stop=(kc == NDC - 1))
                        sig = p_evt.tile([128, HALF], BF16, tag="sig", name="sig")
                        nc.scalar.activation(out=sig, in_=psg, func=AF.Sigmoid,
                                             bias=bt_a[:, pc + NCC:pc + NCC + 1],
                                             scale=1.0)
                        abf = p_evt.tile([128, HALF], BF16, tag="sig",
                                         name="abf")
                        nc.scalar.activation(out=abf, in_=psa, func=AF.Identity,
                                             bias=bt_a[:, pc:pc + 1], scale=1.0)
                        # GLU product on GPSIMD: keeps the DVE free for taps
                        nc.gpsimd.tensor_tensor(
                            out=hp_t[:, PAD + th * HALF:PAD + (th + 1) * HALF],
                            in0=abf, in1=sig, op=OP.mult)
                    hp_e[pc] = hp_t

                def st_tap_pe(pc):
                    if not n_pe:
                        return
                    dgt = p_dg.tile([128, n_pe, 128], MMF, tag="dg", name="dg")
                    nc.sync.dma_start(out=dgt, in_=dram["dw_diag"][pc, :, :, :])
                    pps = []
                    for th in range(2):
                        sl0 = th * HALF
                        pp = ps_sc.tile([128, HALF], F32, tag="sc", name="sc")
                        for t in range(n_pe):
                            nc.tensor.matmul(
                                pp, mm(dgt[:, t, :]),
                                mm(hp_e[pc][:, j0 + t + sl0:
                                            j0 + t + sl0 + HALF]),
                                start=(t == 0), stop=(t == n_pe - 1))
                        pps.append(pp)
                    pe_ps[pc] = pps

                def st_tap_dve(pc):
                    acc = p_acc.tile([128, S], BF16, tag="acc", name="acc")
                    j_first = dve_taps[0]
                    nc.vector.tensor_scalar_mul(
                        out=acc, in0=tap_src(pc, j_first),
                        scalar1=dww[:, pc, j_first:j_first + 1])
                    for j in dve_taps[1:]:
                        nc.vector.scalar_tensor_tensor(
                            out=acc, in0=tap_src(pc, j),
                            scalar=dww[:, pc, j:j + 1], in1=acc,
                            op0=OP.mult, op1=OP.add)
                    acc_d[pc] = acc

                def st_tap_gps(pc):
                    """SG taps: the Scalar engine makes the per-tap products
                    (ACT Copy with per-partition scale — table-set free) and
                    GPSIMD chains the adds. Neither engine is near its
                    budget during conv, and it takes taps off the DVE/PE."""
                    if not gps_taps:
                        return
                    prods = []
                    for j in gps_taps:
                        pg = p_acc.tile([128, S], BF16, tag="acc", name="pg")
                        nc.scalar.mul(out=pg, in_=tap_src(pc, j),
                                      mul=dww[:, pc, j:j + 1])
                        prods.append(pg)
                    accg = prods[0]
                    for pg in prods[1:]:
                        nc.gpsimd.tensor_tensor(out=accg, in0=accg, in1=pg,
                                                op=OP.add)
                    acc_g[pc] = accg

                def st_merge(pc):
                    """Fold GPS acc into DVE acc (GPSIMD cannot touch PSUM),
                    then fold acc into the PE psum as an identity-matmul
                    accumulation (keeps the DVE out of 1x-mode PSUM ops)."""
                    if acc_g[pc] is not None:
                        nc.gpsimd.tensor_tensor(out=acc_d[pc], in0=acc_g[pc],
                                                in1=acc_d[pc], op=OP.add)
                    for th in range(2):
                        sl = slice(th * HALF, (th + 1) * HALF)
                        nc.tensor.matmul(pe_ps[pc][th], mm(ident),
                                         mm(acc_d[pc][:, sl]),
                                         start=False, stop=True)

                def st_silu(pc):
                    """silu(z) = z * sigmoid(z) via the sigmoid table (the
                    silu table lives in a different ACT set; using sigmoid
                    avoids a ~2.7us table swap per chunk). z and sigmoid(z)
                    read the psum on the Scalar engine; the product runs in
                    DVE 2x mode."""
                    ca = p_ca.tile([128, S], MMF, tag="ca", name="ca")
                    for th in range(2):
                        sl = slice(th * HALF, (th + 1) * HALF)
                        zt = p_evt.tile([128, HALF], BF16, tag="zsg",
                                        name="zt", bufs=4)
                        sg = p_evt.tile([128, HALF], BF16, tag="zsg",
                                        name="sg", bufs=4)
                        nc.scalar.activation(out=zt, in_=pe_ps[pc][th],
                                             func=AF.Identity,
                                             bias=dwb[:, pc:pc + 1], scale=1.0)
                        nc.scalar.activation(out=sg, in_=pe_ps[pc][th],
                                             func=AF.Sigmoid,
                                             bias=dwb[:, pc:pc + 1], scale=1.0)
                        nc.vector.tensor_tensor(out=ca[:, sl], in0=zt, in1=sg,
                                                op=OP.mult)
                    ca_t[pc] = ca

                # software-pipelined emission (2-chunk skew: pw1 + GLU of
                # chunk pc+2 overlap the GPS GLU hop and taps of chunk pc)
                st_pw1(0)
                st_pw1(1)
                for pc in range(NCC):
                    if pc + 2 < NCC:
                        st_pw1(pc + 2)
                    st_tap_pe(pc)
                    st_tap_dve(pc)
                    st_tap_gps(pc)
                    st_merge(pc)
                    if pc >= 1:
                        st_silu(pc - 1)
                st_silu(NCC - 1)

                bt2 = load_bias("pw2", NDC)

                def ev2(ps, mc, th, sl):
                    nc.vector.scalar_tensor_tensor(
                        out=x_t[mc][:, sl], in0=ps, scalar=bt2[:, mc:mc + 1],
                        in1=x_t[mc][:, sl], op0=OP.add, op1=OP.add)

                dense_mm("pw2", NCC, NDC, ca_t, ev2)

        # ================= run the block =================
        _mods = os.environ.get("CONF_MODULES", "ffacf")
        if "f" in _mods:
            with nc.named_scope("ff1") if SCOPES else nullcontext():
                ff_module("ff1a", "ff1b")
            shadow_cast()
            dbg_dump("ff1")
        print("built ff1", flush=True)
        if "a" in _mods:
            with nc.named_scope("attn") if SCOPES else nullcontext():
                attn_module()
            shadow_cast()
            dbg_dump("attn")
        print("built attn", flush=True)
        if "c" in _mods:
            with nc.named_scope("conv") if SCOPES else nullcontext():
                conv_module()
            shadow_cast()
            dbg_dump("conv")
        print("built conv", flush=True)
        if _mods.count("f") > 1:
            with nc.named_scope("ff2") if SCOPES else nullcontext():
                ff_module("ff2a", "ff2b")
            shadow_cast()
            dbg_dump("ff2")
        print("built ff2", flush=True)

        # final LN + transpose out (xs is already centered + scaled)
        if SCOPES:
            top.enter_context(nc.named_scope("final"))
        xs_f = ln_stats_rows()
        if not TRIV_FINAL:
            gt = load_bias("outln", NDC)
            bt = p_bias.tile([128, NDC], F32, tag="bias", name="bias")
            nc.sync.dma_start(out=bt, in_=dram["outln_b"].ap())
            for th in range(2):
                sl = slice(th * HALF, (th + 1) * HALF)
                for mc in range(NDC):
                    nc.vector.tensor_scalar(
                        out=xs_f[mc][:, sl], in0=xs_f[mc][:, sl],
                        scalar1=gt[:, mc:mc + 1], scalar2=bt[:, mc:mc + 1],
                        op0=OP.mult, op1=OP.add)
        for tck in range(NTC):
            pt = ps_mm.tile([128, D], MMF, tag="ps", name="ps")
            tsl = slice(tck * 128, (tck + 1) * 128)
            for mc in range(NDC):
                nc.tensor.transpose(out=pt[:, mc * 128:(mc + 1) * 128],
                                    in_=xs_f[mc][:, tsl], identity=ident)
            ob = p_evt.tile([128, D], F32, tag="ob", name="ob")
            nc.scalar.copy(out=ob, in_=pt)
            nc.sync.dma_start(out=y_d[tsl, :], in_=ob)

    _split_excess_waits(nc)
    return nc


def _split_excess_waits(nc, limit=1):
    """This walrus build caps sync-waits per instruction very low; hoist
    excess waits onto single-wait NOPs inserted before the instruction on
    the same engine (same-engine program order preserves the guarantee)."""
    from concourse import mybir
    cnt = 0
    for fn in nc.m.functions:
        for bb in fn.blocks:
            out = []
            for ins in bb.instructions:
                si = getattr(ins, "sync_info", None)
                if si is not None and si.on_wait and len(si.on_wait) > limit:
                    waits = list(si.on_wait)
                    keep = waits[:limit]
                    for w in waits[limit:]:
                        cnt += 1
                        out.append(mybir.InstNoOp(
                            name=f"waitnop_{cnt}",
                            engine=ins.engine,
                            sync_info=mybir.SyncInfo(on_wait=[w],
                                                     on_update=[]),
                        ))
                    si.on_wait = keep
                out.append(ins)
            bb.instructions = out
    return cnt


_CACHE = {}


def _get_program(has_vb, triv_final):
    key = (MM_MODE, GPS_TAPS, PE_TAPS, DBG, has_vb, triv_final)
    if key not in _CACHE:
        build_program._has_vb = has_vb
        build_program._triv_final = triv_final
        _CACHE[key] = build_program()
    return _CACHE[key]


LAST_EXEC_NS = None


def kernel(**inputs):
    global LAST_EXEC_NS
    from concourse.bass_utils import run_bass_kernel_spmd

    w = prep_inputs(inputs)
    has_vb = w.pop("_has_vb")
    triv_final = w.pop("_triv_final")
    nc = _get_program(has_vb, triv_final)

    mdt = _np_mm_dtype()
    x = np.asarray(inputs["x"], np.float32)
    in_maps = []
    for c in range(NCORES):
        m = dict(w)
        m["x"] = np.ascontiguousarray(x[c].T).astype(mdt)
        in_maps.append(m)
    trace = os.environ.get("CONF_TRACE", "0") == "1"
    res = run_bass_kernel_spmd(nc, in_maps, core_ids=list(range(NCORES)),
                               trace=trace)
    LAST_EXEC_NS = res.exec_time_ns
    out = np.stack([res.results[c]["y"] for c in range(NCORES)], 0)
    return out.astype(np.float32)



# revision 34
# speedup vs baseline: 1.3715x; 1.0287x over previous
"""Trainium2 Bass kernel for a Conformer block (B=8, S=1024, D=512).

Sharding: data-parallel over batch — 1 batch element per NeuronCore, 8 cores,
no collectives.

Per-core layout strategy: the residual stream lives in SBUF feature-major
([D, S]) in bf16; every linear layer is then a natural PE matmul with the
stored [in, out] weight as lhsT, all in bf16 (f32 PSUM accumulation) — bf16
keeps the PE out of the fp32-HIGH power-throttle regime. LayerNorm
gains/biases, the attention scale, and BatchNorm are folded into the weights
on the host; x arrives host-transposed ([D, S]) and pre-cast to bf16.
LayerNorm mean/var come from ones-vector matmuls on the PE; mean and rstd
are applied to the matmul input via two K=1 broadcast matmuls plus two
elementwise ops (no augmented-row matmuls); rstd/softmax reciprocals use the
Act-engine Rsqrt/Reciprocal tables (tolerance is loose). Softmax
denominators come for free from a ones column appended to V. The depthwise
conv splits its 31 taps three ways: a middle band as diag-matmul PSUM
accumulation on the otherwise-idle PE, a GPSIMD tail fed by DVE products,
and the rest as DVE scalar_tensor_tensor chains; the conv module is phased
(all GLU sigmoids -> taps -> all SiLUs) to avoid Act table thrash.
"""

import os
import numpy as np

# ---------------- problem constants (hardcoded) ----------------
B, S, D = 8, 1024, 512
H, DH = 8, 64
FFI, CI, KCONV = 1024, 1024, 31
EPS = 1e-5
NCORES = 8
PAD = (KCONV - 1) // 2  # 15
NDC = D // 128    # 4  d-chunks
NTC = S // 128    # 8  t-chunks
HALF = S // 2     # 512

MM_MODE = os.environ.get("CONF_MM_MODE", "bf16")  # bf16 | f32r | f32
SCOPES = os.environ.get("CONF_SCOPES", "1") == "1"
GPS_TAPS = int(os.environ.get("CONF_GPS_TAPS", "0"))  # taps on gpsimd (0:
# the Pool engine rejects AP-scalar ops on this target, so direct GPS taps
# cannot run; the hook remains for a product-fed variant)
PE_TAPS = int(os.environ.get("CONF_PE_TAPS", "22"))  # taps as PE diag matmuls
DBG = os.environ.get("CONF_DEBUG_STAGES", "0") == "1"


# ---------------- tile-framework workaround ----------------
def _patch_tile_drain():
    """This walrus build rejects >1 sync-wait on TPB_CTRL (Drain/NOP)
    instructions; spread the TileContext tail-drain waits across
    single-wait NOPs."""
    import concourse.tile as tile
    from concourse.vector_clock import ScopedClock
    from concourse import mybir

    if getattr(tile.TileContext, "_drain_patched", False):
        return

    def _drain_and_barrier(self, tick_clock, wait_clock):
        nc = self.nc
        carrier = nc.sync.nop(nofuse=True, hint="tail_wait_carrier")
        wait_clock.add_sem_waits(
            carrier.ins, ScopedClock({None: tick_clock.global_clock})
        )
        waits = list(carrier.ins.sync_info.on_wait)
        if len(waits) > 1:
            carrier.ins.sync_info.on_wait = waits[:1]
            for w in waits[1:]:
                nxt = nc.sync.nop(nofuse=True, hint="tail_wait_carrier")
                nxt.ins.sync_info = mybir.SyncInfo(on_wait=[w], on_update=[])
        nc.sync.drain()
        nc.all_engine_barrier()
        assert self.sems is not None
        popped = nc._tile_sem_poison_stack.pop()
        assert popped is self._sem_poison
        nc.clear_and_free_semaphores(list(self.sems.allocated().values()))
        nc.all_engine_barrier()

    tile.TileContext._drain_and_barrier = _drain_and_barrier
    tile.TileContext._drain_patched = True


def _np_mm_dtype():
    import ml_dtypes
    return ml_dtypes.bfloat16 if MM_MODE == "bf16" else np.float32


# ---------------- host-side weight preparation ----------------
def _blob_lhsT(w, nk, nm):
    """[K, M] -> [Mc, 128(p), nk, 128(m)] contiguous (lhsT tile layout)."""
    K, M = w.shape
    assert K == nk * 128 and M == nm * 128
    return np.ascontiguousarray(
        w.reshape(nk, 128, nm, 128).transpose(2, 1, 0, 3)
    ).astype(_np_mm_dtype())


def _blob_bias(b, nm):
    """[M] -> [128, Mc] (per-partition bias columns)."""
    return np.ascontiguousarray(b.reshape(nm, 128).T).astype(np.float32)


def prep_inputs(inp):
    """Fold LN gains/biases, attention scale, BatchNorm, and FF 0.5 scales
    into weights. Returns dict of DRAM arrays shared by all cores."""
    f64 = lambda a: np.asarray(a, np.float64)
    mdt = _np_mm_dtype()
    out = {}

    def ln_matmul_group(pfx, g, lb, w, wb, nm, scale=1.0):
        wg = f64(w) * f64(g)[:, None] * scale
        out[f"{pfx}_w"] = _blob_lhsT(wg, NDC, nm)
        bias = (f64(wb) + f64(lb) @ f64(w)) * scale
        out[f"{pfx}_b"] = _blob_bias(bias, nm)

    # FF1
    ln_matmul_group("ff1a", inp["ff1_ln_g"], inp["ff1_ln_b"],
                    inp["ff1_w1"], inp["ff1_b1"], FFI // 128)
    out["ff1b_w"] = _blob_lhsT(f64(inp["ff1_w2"]) * 0.5, FFI // 128, NDC)
    out["ff1b_b"] = _blob_bias(f64(inp["ff1_b2"]) * 0.5, NDC)

    # attention
    ln_matmul_group("wq", inp["attn_ln_g"], inp["attn_ln_b"],
                    inp["q_w"], inp["q_b"], NDC, scale=DH ** -0.5)
    ln_matmul_group("wk", inp["attn_ln_g"], inp["attn_ln_b"],
                    inp["k_w"], inp["k_b"], NDC)
    # v: rhs layout [kc, p, n]
    wvg = f64(inp["v_w"]) * f64(inp["attn_ln_g"])[:, None]
    out["wv_w"] = np.ascontiguousarray(
        wvg.reshape(NDC, 128, H * DH)
    ).astype(mdt)
    vb = f64(inp["v_b"]) + f64(inp["attn_ln_b"]) @ f64(inp["v_w"])
    out["wv_bias"] = vb.astype(mdt).reshape(1, H * DH)
    out["_has_vb"] = bool(np.abs(vb).max() > 0)
    # o: [Mc, 64(p), H, 128(m)]
    import ml_dtypes
    out["wo_w"] = np.ascontiguousarray(
        f64(inp["o_w"]).reshape(H, DH, NDC, 128).transpose(2, 1, 0, 3)
    ).astype(ml_dtypes.bfloat16)
    out["wo_b"] = _blob_bias(f64(inp["o_b"]), NDC)

    # conv module
    ln_matmul_group("pw1", inp["conv_ln_g"], inp["conv_ln_b"],
                    inp["pw1_w"], inp["pw1_b"], 2 * CI // 128)
    inv = f64(inp["bn_g"]) / np.sqrt(f64(inp["bn_var"]) + EPS)
    dwf = f64(inp["dw_w"])[:, 0, :] * inv[:, None]  # [CI, K]
    out["dw_w"] = np.ascontiguousarray(
        dwf.reshape(CI // 128, 128, KCONV)
    ).astype(np.float32)
    cb = (f64(inp["dw_b"]) - f64(inp["bn_mean"])) * inv + f64(inp["bn_b"])
    out["dw_b"] = _blob_bias(cb, CI // 128)
    out["pw2_w"] = _blob_lhsT(f64(inp["pw2_w"]), CI // 128, NDC)
    out["pw2_b"] = _blob_bias(f64(inp["pw2_b"]), NDC)
    # diag(w[c,j]) lhsT blobs for the PE-matmul taps: [NCC, K, 128, 128]
    n_pe = max(0, min(PE_TAPS, KCONV))
    if n_pe:
        dg = np.zeros((CI // 128, n_pe, 128, 128), np.float64)
        dwr = dwf.reshape(CI // 128, 128, KCONV)
        j0 = (KCONV - n_pe) // 2  # PE takes a middle band of taps
        idx = np.arange(128)
        for pc in range(CI // 128):
            for t in range(n_pe):
                dg[pc, t, idx, idx] = dwr[pc, :, j0 + t]
        out["dw_diag"] = dg.astype(mdt)

    # FF2
    ln_matmul_group("ff2a", inp["ff2_ln_g"], inp["ff2_ln_b"],
                    inp["ff2_w1"], inp["ff2_b1"], FFI // 128)
    out["ff2b_w"] = _blob_lhsT(f64(inp["ff2_w2"]) * 0.5, FFI // 128, NDC)
    out["ff2b_b"] = _blob_bias(f64(inp["ff2_b2"]) * 0.5, NDC)

    # final LN
    out["outln_g"] = _blob_bias(f64(inp["out_ln_g"]), NDC)
    out["outln_b"] = _blob_bias(f64(inp["out_ln_b"]), NDC)
    out["_triv_final"] = bool(
        np.allclose(inp["out_ln_g"], 1.0) and np.allclose(inp["out_ln_b"], 0.0)
    )
    out["ident"] = np.eye(128, dtype=mdt)
    out["ones_c"] = np.ones((128, 128), dtype=mdt)
    return out


# ---------------- kernel builder ----------------
def build_program():
    _patch_tile_drain()
    import concourse.bass as bass
    import concourse.tile as tile
    from concourse import mybir
    from contextlib import ExitStack, nullcontext

    dt = mybir.dt
    AF = mybir.ActivationFunctionType
    OP = mybir.AluOpType
    F32 = dt.float32
    BF16 = dt.bfloat16
    F8 = dt.float8e4
    DR = mybir.MatmulPerfMode.DoubleRow

    MMF = {"bf16": dt.bfloat16, "f32r": dt.float32r,
           "f32": dt.float32}[MM_MODE]

    def mm(ap):
        return ap

    nc = bass.Bass("TRN2", target_bir_lowering=False, debug=False)

    def act_unsafe(out, in_, func, bias=0.0, scale=1.0):
        """Emit an InstActivation bypassing the Reciprocal/Rsqrt accuracy
        guard in bass (tolerance here is 2e-2; the table approximation is
        fine and ~5x faster than the DVE multi-pass reciprocal)."""
        eng = nc.scalar
        inputs = [eng.lower_ap(in_)]
        for arg in (bias, scale, 0.0):
            if isinstance(arg, bass.AP):
                inputs.append(eng.lower_ap(arg))
            else:
                inputs.append(mybir.ImmediateValue(dtype=mybir.dt.float32,
                                                   value=float(arg)))
        return eng.add_instruction(mybir.InstActivation(
            name=nc.get_next_instruction_name(),
            func=func,
            ins=inputs,
            outs=[eng.lower_ap(out)],
        ))

    # ---- DRAM declarations ----
    x_d = nc.dram_tensor("x", [D, S], MMF, kind="ExternalInput")
    y_d = nc.dram_tensor("y", [S, D], F32, kind="ExternalOutput")
    dram = {}

    def din(name, shape, dtp=None):
        dram[name] = nc.dram_tensor(name, list(shape), dtp or F32,
                                    kind="ExternalInput")
        return dram[name]

    def dinm(name, shape):
        return din(name, shape, MMF)

    dinm("ident", [128, 128])
    dinm("ones_c", [128, 128])
    for pfx, nm in [("ff1a", FFI // 128), ("wq", NDC), ("wk", NDC),
                    ("pw1", 2 * CI // 128), ("ff2a", FFI // 128)]:
        dinm(f"{pfx}_w", [nm, 128, NDC, 128])
        din(f"{pfx}_b", [128, nm])
    for pfx, nk, nm in [("ff1b", FFI // 128, NDC), ("pw2", CI // 128, NDC),
                        ("ff2b", FFI // 128, NDC)]:
        dinm(f"{pfx}_w", [nm, 128, nk, 128])
        din(f"{pfx}_b", [128, nm])
    dinm("wv_w", [NDC, 128, H * DH])
    dinm("wv_bias", [1, H * DH])
    N_PE_TAPS = max(0, min(PE_TAPS, KCONV))
    if N_PE_TAPS:
        dinm("dw_diag", [CI // 128, N_PE_TAPS, 128, 128])
    din("wo_w", [NDC, DH, H, 128], BF16)
    din("wo_b", [128, NDC])
    din("dw_w", [CI // 128, 128, KCONV])
    din("dw_b", [128, CI // 128])
    din("outln_g", [128, NDC])
    din("outln_b", [128, NDC])

    dbg_d = {}
    if DBG:
        for s_ in ["ff1", "attn", "conv", "ff2"]:
            dbg_d[s_] = nc.dram_tensor(f"dbg_{s_}", [D, S], MMF,
                                       kind="ExternalOutput")

    HAS_VB = build_program._has_vb
    TRIV_FINAL = build_program._triv_final

    with tile.TileContext(nc) as tc, ExitStack() as top:
        top.enter_context(nc.allow_low_precision(
            reason="bf16 compute is intentional"))
        # ---- global pools ----
        p_x = top.enter_context(tc.tile_pool(name="p_x", bufs=1))
        p_const = top.enter_context(tc.tile_pool(name="p_const", bufs=1))
        p_rows = top.enter_context(tc.tile_pool(name="p_rows", bufs=2))
        p_sq = top.enter_context(tc.tile_pool(name="p_sq", bufs=2))
        p_w = top.enter_context(tc.tile_pool(name="p_w", bufs=3))
        p_wsm = top.enter_context(tc.tile_pool(name="p_wsm", bufs=2))
        p_bias = top.enter_context(tc.tile_pool(name="p_bias", bufs=2))
        p_xs = top.enter_context(tc.tile_pool(name="p_xs", bufs=4))
        p_evt = top.enter_context(tc.tile_pool(name="p_evt", bufs=2))
        # PSUM budget is 8 banks. ps_sc feeds the two MM->ACT ping-pong
        # streams (attention scores->exp, conv tap psums) with enough slots
        # that the producer never waits on the consumer's semaphore
        # round-trip; ps_mm covers everything else; ps_st holds one LN stat.
        ps_mm = top.enter_context(tc.tile_pool(name="ps_mm", bufs=3, space="PSUM"))
        ps_sc = top.enter_context(tc.tile_pool(name="ps_sc", bufs=4, space="PSUM"))
        ps_st = top.enter_context(tc.tile_pool(name="ps_st", bufs=1, space="PSUM"))

        ident = p_const.tile([128, 128], MMF, tag="ident", name="ident")
        nc.sync.dma_start(out=ident, in_=dram["ident"].ap())
        ones = p_const.tile([128, 128], MMF, tag="ones", name="ones")
        nc.sync.dma_start(out=ones, in_=dram["ones_c"].ap())
        epst = p_const.tile([128, 1], F32, tag="epst", name="epst")
        nc.vector.memset(epst, EPS)
        ones8 = p_const.tile([128, 2, 64], F8, tag="ones8", name="ones8")
        nc.vector.memset(ones8, 1.0)

        # ---- load x (already feature-major + cast on host) ----
        x_t = [p_x.tile([128, S], MMF, tag=f"x{i}", name=f"x{i}") for i in range(NDC)]
        xb_t = x_t
        for mc in range(NDC):
            nc.sync.dma_start(out=x_t[mc], in_=x_d[mc * 128:(mc + 1) * 128, :])

        def shadow_cast():
            pass

        # ---- helpers ----
        def ln_stats_rows():
            """LN over feature axis of x_t. Returns xs tiles with
            xs = (x - mean) * rstd (feature-major, centered + scaled)."""
            rstd_b = p_rows.tile([1, S], MMF, tag="rstdb", name="rstdb")
            mrow = p_rows.tile([1, S], MMF, tag="mrow", name="mrow")
            c1 = 1.0 / D
            for th in range(2):
                sl = slice(th * HALF, (th + 1) * HALF)
                s1 = ps_st.tile([1, HALF], F32, tag="st", name="st")
                s2 = ps_mm.tile([1, HALF], F32, tag="ps", name="st2")
                for kc in range(NDC):
                    nc.tensor.matmul(s1, mm(ones[:, 0:1]),
                                     mm(xb_t[kc][:, sl]),
                                     start=(kc == 0), stop=(kc == NDC - 1))
                for kc in range(NDC):
                    sq = p_sq.tile([128, HALF], MMF, tag="sq", name="sq")
                    nc.scalar.square(out=sq, in_=xb_t[kc][:, sl])
                    nc.tensor.matmul(s2, mm(ones[:, 0:1]), mm(sq),
                                     start=(kc == 0), stop=(kc == NDC - 1))
                mean_s = p_rows.tile([1, HALF], F32, tag="tmp",
                                     name="mean_s", bufs=4)
                nc.vector.tensor_scalar_mul(out=mean_s, in0=s1, scalar1=c1)
                msq = p_rows.tile([1, HALF], F32, tag="tmp", name="msq", bufs=4)
                nc.vector.tensor_tensor(out=msq, in0=mean_s, in1=mean_s,
                                        op=OP.mult)
                vpe = p_rows.tile([1, HALF], F32, tag="tmp", name="vpe", bufs=4)
                nc.vector.scalar_tensor_tensor(
                    out=vpe, in0=s2, scalar=c1, in1=msq,
                    op0=OP.mult, op1=OP.subtract)
                # rsqrt via exp(-0.5*ln(v+eps)): ln+exp share one ACT table
                # set (natural_log_exp_and_others), so no table swap against
                # the attention exp / softmax reciprocal path.
                lnv = p_rows.tile([1, HALF], F32, tag="tmp", name="lnv",
                                  bufs=4)
                nc.scalar.activation(out=lnv, in_=vpe, func=AF.Ln,
                                     bias=epst[0:1, :], scale=1.0)
                nc.scalar.activation(out=rstd_b[:, sl], in_=lnv, func=AF.Exp,
                                     bias=0.0, scale=-0.5)
                nc.vector.scalar_tensor_tensor(
                    out=mrow[:, sl], in0=mean_s, scalar=-1.0,
                    in1=rstd_b[:, sl], op0=OP.mult, op1=OP.mult)
            # xs = x * bcast(rstd) + bcast(-mean*rstd); the broadcast tiles
            # are copied to SBUF once (GPSIMD cannot read PSUM) and the
            # per-chunk applies are split DVE/GPSIMD to keep the DVE free.
            xs = [p_xs.tile([128, S], MMF, tag="xs", name="xs") for _ in range(NDC)]
            for th in range(2):
                sl = slice(th * HALF, (th + 1) * HALF)
                pb = ps_mm.tile([128, HALF], F32, tag="ps", name="ps")
                nc.tensor.matmul(pb, mm(ones[0:1, :]), mm(rstd_b[:, sl]),
                                 start=True, stop=True)
                pm = ps_mm.tile([128, HALF], F32, tag="ps", name="ps")
                nc.tensor.matmul(pm, mm(ones[0:1, :]), mm(mrow[:, sl]),
                                 start=True, stop=True)
                pbs = p_evt.tile([128, HALF], BF16, tag="zsg", name="pbs",
                                 bufs=4)
                pms = p_evt.tile([128, HALF], BF16, tag="zsg", name="pms",
                                 bufs=4)
                nc.scalar.copy(out=pbs, in_=pb)
                nc.scalar.copy(out=pms, in_=pm)
                for kc in range(NDC):
                    eng = nc.vector if kc < 2 else nc.gpsimd
                    eng.tensor_tensor(out=xs[kc][:, sl],
                                      in0=x_t[kc][:, sl], in1=pbs,
                                      op=OP.mult)
                    eng.tensor_tensor(out=xs[kc][:, sl],
                                      in0=xs[kc][:, sl], in1=pms,
                                      op=OP.add)
            return xs

        def load_bias(pfx, nm):
            bt = p_bias.tile([128, nm], F32, tag="bias", name="bias")
            nc.sync.dma_start(out=bt, in_=dram[f"{pfx}_b"].ap())
            return bt

        def dense_mm(pfx, nk, nm, rhs_tiles, evict):
            """plain contraction over nk chunks of rhs_tiles."""
            for mc in range(nm):
                wt = p_w.tile([128, nk, 128], MMF, tag="w", name="w")
                nc.sync.dma_start(out=wt, in_=dram[f"{pfx}_w"][mc, :, :, :])
                for th in range(2):
                    sl = slice(th * HALF, (th + 1) * HALF)
                    ps = ps_mm.tile([128, HALF], F32, tag="ps", name="ps")
                    for kc in range(nk):
                        nc.tensor.matmul(ps, mm(wt[:, kc, :]),
                                         mm(rhs_tiles[kc][:, sl]),
                                         start=(kc == 0), stop=(kc == nk - 1))
                    evict(ps, mc, th, sl)

        def dbg_dump(name):
            if DBG:
                for mc in range(NDC):
                    nc.sync.dma_start(
                        out=dbg_d[name][mc * 128:(mc + 1) * 128, :],
                        in_=x_t[mc])

        # ================= FF module =================
        def ff_module(pa, pb):
            xs = ln_stats_rows()
            with tc.tile_pool(name="p_h", bufs=FFI // 128) as p_h:
                bt1 = load_bias(pa, FFI // 128)
                h = [p_h.tile([128, S], MMF, tag="h", name="h") for _ in range(FFI // 128)]

                def ev1(ps, mc, th, sl):
                    nc.scalar.activation(out=h[mc][:, sl], in_=ps, func=AF.Silu,
                                         bias=bt1[:, mc:mc + 1], scale=1.0)

                dense_mm(pa, NDC, FFI // 128, xs, ev1)
                bt2 = load_bias(pb, NDC)

                def ev2(ps, mc, th, sl):
                    nc.vector.scalar_tensor_tensor(
                        out=x_t[mc][:, sl], in0=ps, scalar=bt2[:, mc:mc + 1],
                        in1=x_t[mc][:, sl], op0=OP.add, op1=OP.add)

                dense_mm(pb, FFI // 128, NDC, h, ev2)

        # ================= attention =================
        def attn_module():
            xs = ln_stats_rows()
            with ExitStack() as ph:
                p_qk = ph.enter_context(tc.tile_pool(name="p_qk", bufs=8))
                p_v = ph.enter_context(tc.tile_pool(name="p_v", bufs=NTC))
                p_exp = ph.enter_context(tc.tile_pool(name="p_exp", bufs=24))
                p_ao = ph.enter_context(tc.tile_pool(name="p_ao", bufs=H))
                p_wv = ph.enter_context(tc.tile_pool(name="p_wv", bufs=1))

                q_t = [p_qk.tile([128, S], BF16, tag="qk", name="qk") for _ in range(NDC)]
                k_t = [p_qk.tile([128, S], BF16, tag="qk", name="qk") for _ in range(NDC)]
                btq = load_bias("wq", NDC)
                btk = load_bias("wk", NDC)

                def evq(ps, mc, th, sl):
                    nc.vector.tensor_scalar_add(out=q_t[mc][:, sl], in0=ps,
                                                scalar1=btq[:, mc:mc + 1])

                def evk(ps, mc, th, sl):
                    nc.vector.tensor_scalar_add(out=k_t[mc][:, sl], in0=ps,
                                                scalar1=btk[:, mc:mc + 1])

                dense_mm("wq", NDC, NDC, xs, evq)
                dense_mm("wk", NDC, NDC, xs, evk)

                # v (token-major, with ones column per head)
                wv = p_wv.tile([128, NDC, H * DH], MMF, tag="wv", name="wv")
                nc.sync.dma_start(out=wv, in_=dram["wv_w"].ap().rearrange(
                    "k p n -> p k n"))
                if HAS_VB:
                    wvb = p_wv.tile([1, H * DH], MMF, tag="wvb", name="wvb")
                    nc.sync.dma_start(out=wvb, in_=dram["wv_bias"].ap())
                # v (token-major, with ones column per head)
                v_t = []
                for tck in range(NTC):
                    vt = p_v.tile([128, H, DH + 1], BF16, tag="v", name="v")
                    nc.vector.memset(vt[:, :, DH:DH + 1], 1.0)
                    pv = ps_mm.tile([128, H * DH], F32, tag="ps", name="ps")
                    tsl = slice(tck * 128, (tck + 1) * 128)
                    for kc in range(NDC):
                        nc.tensor.matmul(pv, mm(xs[kc][:, tsl]),
                                         mm(wv[:, kc, :]),
                                         start=(kc == 0),
                                         stop=(kc == NDC - 1 and not HAS_VB))
                    if HAS_VB:
                        nc.tensor.matmul(pv, mm(ones[0:1, :]), mm(wvb),
                                         start=False, stop=True)
                    nc.vector.tensor_copy(
                        out=vt[:, :, 0:DH],
                        in_=pv.rearrange("p (h d) -> p h d", h=H))
                    v_t.append(vt)

                # scores -> exp -> AV -> normalize, software-pipelined with a
                # one-head skew: the PE runs head h+1's score matmuls while
                # the Scalar engine exps head h, so neither engine waits and
                # the PE never idles past the HAM re-throttle window.
                ao_t = [None] * H
                e_heads = [None] * H

                def emit_scores(h_):
                    hp, sub = h_ // 2, h_ % 2
                    base = sub * 64
                    e_t = []
                    for ktc in range(NTC):
                        et = p_exp.tile([128, S], BF16, tag="exp", name="exp")
                        ksl = slice(ktc * 128, (ktc + 1) * 128)
                        for th in range(2):
                            sl = slice(th * HALF, (th + 1) * HALF)
                            pss = ps_sc.tile([128, HALF], F32, tag="sc",
                                             name="sc")
                            nc.tensor.matmul(
                                pss,
                                mm(k_t[hp][base:base + 64, ksl]),
                                mm(q_t[hp][base:base + 64, sl]),
                                start=True, stop=True,
                                tile_position=(base, 0))
                            nc.scalar.activation(out=et[:, sl], in_=pss,
                                                 func=AF.Exp)
                        e_t.append(et)
                    e_heads[h_] = e_t

                pavs_t = [None] * H

                def emit_av_mm(h_):
                    """AV matmuls; pav is copied to SBUF immediately so the
                    psum bank frees within ~0.5us."""
                    e_t = e_heads[h_]
                    pavs = [None, None]
                    for th in range(2):
                        sl = slice(th * HALF, (th + 1) * HALF)
                        pav = ps_mm.tile([65, HALF], F32, tag="ps", name="ps")
                        for ktc in range(NTC):
                            nc.tensor.matmul(pav, mm(v_t[ktc][:, h_, :]),
                                             mm(e_t[ktc][:, sl]),
                                             start=(ktc == 0),
                                             stop=(ktc == NTC - 1))
                        pv_s = p_evt.tile([65, HALF], BF16, tag="pavs",
                                          name="pavs", bufs=6)
                        nc.vector.tensor_copy(out=pv_s, in_=pav)
                        pavs[th] = pv_s
                    e_heads[h_] = None
                    pavs_t[h_] = pavs

                def emit_norm(h_):
                    at = p_ao.tile([64, S], BF16, tag="ao", name="ao")
                    for th in range(2):
                        sl = slice(th * HALF, (th + 1) * HALF)
                        pv_s = pavs_t[h_][th]
                        rrb = p_rows.tile([1, HALF], MMF, tag="tmp2", name="rrb",
                                          bufs=4)
                        # 1/x via exp(-ln(x)): stays in the exp table set, so
                        # the softmax loop never swaps ACT tables.
                        lnd = p_rows.tile([1, HALF], F32, tag="tmp2",
                                          name="lnd", bufs=4)
                        nc.scalar.activation(out=lnd, in_=pv_s[64:65, :],
                                             func=AF.Ln)
                        nc.scalar.activation(out=rrb, in_=lnd, func=AF.Exp,
                                             bias=0.0, scale=-1.0)
                        prb = ps_mm.tile([64, HALF], F32, tag="ps", name="ps")
                        nc.tensor.matmul(prb, mm(ones[0:1, 0:64]), mm(rrb),
                                         start=True, stop=True)
                        rbs = p_evt.tile([64, HALF], BF16, tag="rbs",
                                         name="rbs")
                        nc.vector.tensor_copy(out=rbs, in_=prb)
                        nc.vector.tensor_tensor(out=at[:, sl],
                                                in0=pv_s[0:64, :], in1=rbs,
                                                op=OP.mult)
                    pavs_t[h_] = None
                    ao_t[h_] = at

                # pipeline: AV of head h rides inside the exp stream of head
                # h+1; score matmuls keep a two-head lead; the normalize
                # chain (which blocks on the Scalar exp batch) trails by one
                # head so it never stalls the PE score/AV streams.
                emit_scores(0)
                emit_scores(1)
                for h_ in range(H):
                    emit_av_mm(h_)
                    if h_ + 2 < H:
                        emit_scores(h_ + 2)
                    if h_ >= 1:
                        emit_norm(h_ - 1)
                emit_norm(H - 1)

                # o-projection + residual
                bto = load_bias("wo", NDC)
                for mc in range(NDC):
                    wo = p_wsm.tile([DH, H, 128], BF16, tag="wo", name="wo")
                    nc.sync.dma_start(out=wo, in_=dram["wo_w"][mc, :, :, :])
                    for th in range(2):
                        sl = slice(th * HALF, (th + 1) * HALF)
                        ps = ps_mm.tile([128, HALF], F32, tag="ps", name="ps")
                        for h_ in range(H):
                            nc.tensor.matmul(ps, mm(wo[:, h_, :]),
                                             mm(ao_t[h_][:, sl]),
                                             start=(h_ == 0),
                                             stop=(h_ == H - 1))
                        nc.vector.scalar_tensor_tensor(
                            out=x_t[mc][:, sl], in0=ps,
                            scalar=bto[:, mc:mc + 1], in1=x_t[mc][:, sl],
                            op0=OP.add, op1=OP.add)

        # ================= conv module =================
        # Tap schedule: a middle band of PE_TAPS runs as diag-matmul PSUM
        # accumulation; GPS_TAPS run as direct scalar_tensor_tensor on
        # GPSIMD; the rest are DVE scalar_tensor_tensor chains. All DVE tap
        # reads are kept 4B-aligned (even-j from hp_e, odd-j from a
        # one-element-shifted shadow hp_o made on the Scalar engine) so the
        # DVE runs in its 2x 16-bit mode. Emission is software-pipelined
        # per channel chunk so the PE never idles long enough to
        # re-throttle (HAM 3.4us window).
        def conv_module():
            xs = ln_stats_rows()
            NCC = CI // 128
            SHP = S + 2 * PAD
            n_pe = max(0, min(PE_TAPS, KCONV - 2))
            j0 = (KCONV - n_pe) // 2
            rest = [j for j in range(KCONV)
                    if not (j0 <= j < j0 + n_pe)]
            n_gps = max(0, min(GPS_TAPS, len(rest) - 1))
            gps_taps = rest[len(rest) - n_gps:] if n_gps else []
            dve_taps = rest[:len(rest) - n_gps]
            with ExitStack() as ph:
                p_hp = ph.enter_context(tc.tile_pool(name="p_hp", bufs=NCC))
                p_ca = ph.enter_context(tc.tile_pool(name="p_ca", bufs=NCC))
                p_acc = ph.enter_context(tc.tile_pool(name="p_acc",
                         bufs=4 if not GPS_TAPS else 2 * (GPS_TAPS + 1)))
                p_dg = ph.enter_context(tc.tile_pool(name="p_dg", bufs=4))
                p_dw = ph.enter_context(tc.tile_pool(name="p_dw", bufs=2))

                bt_a = load_bias("pw1", 2 * CI // 128)  # [128, 16]
                dwb = load_bias("dw", NCC)
                dww = p_dw.tile([128, NCC, KCONV], F32, tag="dww", name="dww")
                nc.sync.dma_start(out=dww, in_=dram["dw_w"].ap().rearrange(
                    "c p k -> p c k"))

                hp_e = [None] * NCC
                pe_ps = [None] * NCC
                acc_d = [None] * NCC
                acc_g = [None] * NCC
                ca_t = [None] * NCC

                def tap_src(pc, j):
                    return hp_e[pc][:, j:j + S]

                def st_pw1(pc):
                    """pw1 matmuls + sigmoid + GLU -> hp_e[pc]."""
                    hp_t = p_hp.tile([128, SHP], BF16, tag="hp", name="hp")
                    nc.vector.memset(hp_t[:, 0:PAD], 0.0)
                    nc.vector.memset(hp_t[:, PAD + S:], 0.0)
                    wt_a = p_w.tile([128, NDC, 128], MMF, tag="w", name="w")
                    nc.sync.dma_start(out=wt_a, in_=dram["pw1_w"][pc, :, :, :])
                    wt_g = p_w.tile([128, NDC, 128], MMF, tag="w", name="w")
                    nc.sync.dma_start(out=wt_g,
                                      in_=dram["pw1_w"][pc + NCC, :, :, :])
                    for th in range(2):
                        sl = slice(th * HALF, (th + 1) * HALF)
                        psa = ps_mm.tile([128, HALF], F32, tag="ps", name="ps")
                        psg = ps_mm.tile([128, HALF], F32, tag="ps", name="ps")
                        for kc in range(NDC):
                            nc.tensor.matmul(psg, mm(wt_g[:, kc, :]),
                                             mm(xs[kc][:, sl]),
                                             start=(kc == 0),
                                             stop=(kc == NDC - 1))
                        for kc in range(NDC):
                            nc.tensor.matmul(psa, mm(wt_a[:, kc, :]),
                                             mm(xs[kc][:, sl]),
                                             start=(kc == 0),
                                             stop=(kc == NDC - 1))
                        sig = p_evt.tile([128, HALF], BF16, tag="sig", name="sig")
                        nc.scalar.activation(out=sig, in_=psg, func=AF.Sigmoid,
                                             bias=bt_a[:, pc + NCC:pc + NCC + 1],
                                             scale=1.0)
                        abf = p_evt.tile([128, HALF], BF16, tag="sig",
                                         name="abf")
                        nc.scalar.activation(out=abf, in_=psa, func=AF.Identity,
                                             bias=bt_a[:, pc:pc + 1], scale=1.0)
                        # GLU product on GPSIMD: keeps the DVE free for taps
                        nc.gpsimd.tensor_tensor(
                            out=hp_t[:, PAD + th * HALF:PAD + (th + 1) * HALF],
                            in0=abf, in1=sig, op=OP.mult)
                    hp_e[pc] = hp_t

                def st_tap_pe(pc):
                    if not n_pe:
                        return
                    dgt = p_dg.tile([128, n_pe, 128], MMF, tag="dg", name="dg")
                    nc.sync.dma_start(out=dgt, in_=dram["dw_diag"][pc, :, :, :])
                    pps = []
                    for th in range(2):
                        sl0 = th * HALF
                        pp = ps_sc.tile([128, HALF], F32, tag="sc", name="sc")
                        for t in range(n_pe):
                            nc.tensor.matmul(
                                pp, mm(dgt[:, t, :]),
                                mm(hp_e[pc][:, j0 + t + sl0:
                                            j0 + t + sl0 + HALF]),
                                start=(t == 0), stop=(t == n_pe - 1))
                        pps.append(pp)
                    pe_ps[pc] = pps

                def st_tap_dve(pc):
                    acc = p_acc.tile([128, S], BF16, tag="acc", name="acc")
                    j_first = dve_taps[0]
                    nc.vector.tensor_scalar_mul(
                        out=acc, in0=tap_src(pc, j_first),
                        scalar1=dww[:, pc, j_first:j_first + 1])
                    for j in dve_taps[1:]:
                        nc.vector.scalar_tensor_tensor(
                            out=acc, in0=tap_src(pc, j),
                            scalar=dww[:, pc, j:j + 1], in1=acc,
                            op0=OP.mult, op1=OP.add)
                    acc_d[pc] = acc

                def st_tap_gps(pc):
                    """SG taps: the Scalar engine makes the per-tap products
                    (ACT Copy with per-partition scale — table-set free) and
                    GPSIMD chains the adds. Neither engine is near its
                    budget during conv, and it takes taps off the DVE/PE."""
                    if not gps_taps:
                        return
                    prods = []
                    for j in gps_taps:
                        pg = p_acc.tile([128, S], BF16, tag="acc", name="pg")
                        nc.scalar.mul(out=pg, in_=tap_src(pc, j),
                                      mul=dww[:, pc, j:j + 1])
                        prods.append(pg)
                    accg = prods[0]
                    for pg in prods[1:]:
                        nc.gpsimd.tensor_tensor(out=accg, in0=accg, in1=pg,
                                                op=OP.add)
                    acc_g[pc] = accg

                def st_merge(pc):
                    """Fold GPS acc into DVE acc (GPSIMD cannot touch PSUM),
                    then fold acc into the PE psum as an identity-matmul
                    accumulation (keeps the DVE out of 1x-mode PSUM ops)."""
                    if acc_g[pc] is not None:
                        nc.gpsimd.tensor_tensor(out=acc_d[pc], in0=acc_g[pc],
                                                in1=acc_d[pc], op=OP.add)
                    for th in range(2):
                        sl = slice(th * HALF, (th + 1) * HALF)
                        nc.tensor.matmul(pe_ps[pc][th], mm(ident),
                                         mm(acc_d[pc][:, sl]),
                                         start=False, stop=True)

                def st_silu(pc):
                    """silu(z) = z * sigmoid(z) via the sigmoid table (the
                    silu table lives in a different ACT set; using sigmoid
                    avoids a ~2.7us table swap per chunk). z and sigmoid(z)
                    read the psum on the Scalar engine; the product runs in
                    DVE 2x mode."""
                    ca = p_ca.tile([128, S], MMF, tag="ca", name="ca")
                    for th in range(2):
                        sl = slice(th * HALF, (th + 1) * HALF)
                        zt = p_evt.tile([128, HALF], BF16, tag="zsg",
                                        name="zt", bufs=4)
                        sg = p_evt.tile([128, HALF], BF16, tag="zsg",
                                        name="sg", bufs=4)
                        nc.scalar.activation(out=zt, in_=pe_ps[pc][th],
                                             func=AF.Identity,
                                             bias=dwb[:, pc:pc + 1], scale=1.0)
                        nc.scalar.activation(out=sg, in_=pe_ps[pc][th],
                                             func=AF.Sigmoid,
                                             bias=dwb[:, pc:pc + 1], scale=1.0)
                        nc.vector.tensor_tensor(out=ca[:, sl], in0=zt, in1=sg,
                                                op=OP.mult)
                    ca_t[pc] = ca

                # software-pipelined emission (2-chunk skew: pw1 + GLU of
                # chunk pc+2 overlap the GPS GLU hop and taps of chunk pc)
                st_pw1(0)
                st_pw1(1)
                for pc in range(NCC):
                    if pc + 2 < NCC:
                        st_pw1(pc + 2)
                    st_tap_pe(pc)
                    st_tap_dve(pc)
                    st_tap_gps(pc)
                    st_merge(pc)
                    if pc >= 1:
                        st_silu(pc - 1)
                st_silu(NCC - 1)

                bt2 = load_bias("pw2", NDC)

                def ev2(ps, mc, th, sl):
                    nc.vector.scalar_tensor_tensor(
                        out=x_t[mc][:, sl], in0=ps, scalar=bt2[:, mc:mc + 1],
                        in1=x_t[mc][:, sl], op0=OP.add, op1=OP.add)

                dense_mm("pw2", NCC, NDC, ca_t, ev2)

        # ================= run the block =================
        _mods = os.environ.get("CONF_MODULES", "ffacf")
        if "f" in _mods:
            with nc.named_scope("ff1") if SCOPES else nullcontext():
                ff_module("ff1a", "ff1b")
            shadow_cast()
            dbg_dump("ff1")
        print("built ff1", flush=True)
        if "a" in _mods:
            with nc.named_scope("attn") if SCOPES else nullcontext():
                attn_module()
            shadow_cast()
            dbg_dump("attn")
        print("built attn", flush=True)
        if "c" in _mods:
            with nc.named_scope("conv") if SCOPES else nullcontext():
                conv_module()
            shadow_cast()
            dbg_dump("conv")
        print("built conv", flush=True)
        if _mods.count("f") > 1:
            with nc.named_scope("ff2") if SCOPES else nullcontext():
                ff_module("ff2a", "ff2b")
            shadow_cast()
            dbg_dump("ff2")
        print("built ff2", flush=True)

        # final LN + transpose out (xs is already centered + scaled)
        if SCOPES:
            top.enter_context(nc.named_scope("final"))
        xs_f = ln_stats_rows()
        if not TRIV_FINAL:
            gt = load_bias("outln", NDC)
            bt = p_bias.tile([128, NDC], F32, tag="bias", name="bias")
            nc.sync.dma_start(out=bt, in_=dram["outln_b"].ap())
            for th in range(2):
                sl = slice(th * HALF, (th + 1) * HALF)
                for mc in range(NDC):
                    nc.vector.tensor_scalar(
                        out=xs_f[mc][:, sl], in0=xs_f[mc][:, sl],
                        scalar1=gt[:, mc:mc + 1], scalar2=bt[:, mc:mc + 1],
                        op0=OP.mult, op1=OP.add)
        for tck in range(NTC):
            pt = ps_mm.tile([128, D], MMF, tag="ps", name="ps")
            tsl = slice(tck * 128, (tck + 1) * 128)
            for mc in range(NDC):
                nc.tensor.transpose(out=pt[:, mc * 128:(mc + 1) * 128],
                                    in_=xs_f[mc][:, tsl], identity=ident)
            ob = p_evt.tile([128, D], F32, tag="ob", name="ob")
            nc.scalar.copy(out=ob, in_=pt)
            nc.sync.dma_start(out=y_d[tsl, :], in_=ob)

    _split_excess_waits(nc)
    return nc


def _split_excess_waits(nc, limit=1):
    """This walrus build caps sync-waits per instruction very low; hoist
    excess waits onto single-wait NOPs inserted before the instruction on
    the same engine (same-engine program order preserves the guarantee)."""
    from concourse import mybir
    cnt = 0
    for fn in nc.m.functions:
        for bb in fn.blocks:
            out = []
            for ins in bb.instructions:
                si = getattr(ins, "sync_info", None)
                if si is not None and si.on_wait and len(si.on_wait) > limit:
                    waits = list(si.on_wait)
                    keep = waits[:limit]
                    for w in waits[limit:]:
                        cnt += 1
                        out.append(mybir.InstNoOp(
                            name=f"waitnop_{cnt}",
                            engine=ins.engine,
                            sync_info=mybir.SyncInfo(on_wait=[w],
                                                     on_update=[]),
                        ))
                    si.on_wait = keep
                out.append(ins)
            bb.instructions = out
    return cnt


_CACHE = {}


def _get_program(has_vb, triv_final):
    key = (MM_MODE, GPS_TAPS, PE_TAPS, DBG, has_vb, triv_final)
    if key not in _CACHE:
        build_program._has_vb = has_vb
        build_program._triv_final = triv_final
        _CACHE[key] = build_program()
    return _CACHE[key]


LAST_EXEC_NS = None


def kernel(**inputs):
    global LAST_EXEC_NS
    from concourse.bass_utils import run_bass_kernel_spmd

    w = prep_inputs(inputs)
    has_vb = w.pop("_has_vb")
    triv_final = w.pop("_triv_final")
    nc = _get_program(has_vb, triv_final)

    mdt = _np_mm_dtype()
    x = np.asarray(inputs["x"], np.float32)
    in_maps = []
    for c in range(NCORES):
        m = dict(w)
        m["x"] = np.ascontiguousarray(x[c].T).astype(mdt)
        in_maps.append(m)
    trace = os.environ.get("CONF_TRACE", "0") == "1"
    res = run_bass_kernel_spmd(nc, in_maps, core_ids=list(range(NCORES)),
                               trace=trace)
    LAST_EXEC_NS = res.exec_time_ns
    out = np.stack([res.results[c]["y"] for c in range(NCORES)], 0)
    return out.astype(np.float32)

